# revision 1
# baseline (speedup 1.0000x reference)
"""Distributed Trainium2 kernel for nn_Attention_18562848653411.

Reference model: fc_in -> LayerNorm -> 4 sequential "refinement heads"
(qkv matmul + gelu, scores=q@k^T/C, att=scores@v, softmax over channels,
proj + gelu, residual with head-0 output) -> fc_out + PoseEncoding.

Key algebra: softmax comes AFTER att = scores@v, so per head
att^T = (v^T k) q^T / C = M q^T / C with M = v^T k a [C,C] matrix that
is a sum over sequence positions. No S x S scores are ever formed.

Sharding (8 NeuronCores): core c handles batch b=c//2, sequence half
c%2. All weights replicated, fp8e4 (x16 host scale); every matmul is a
DoubleRow fp8 instruction (2 k-tiles / 256-deep contraction, f32 PSUM,
<=512 f32 output columns per instruction). Per head each core computes
k/v and M_own = v_own^T k_own over its OWN 1024 rows only and exchanges
M_own (1 MB fp8) with its pair partner through pair-shared HBM
(addr_space="Shared"): write M_own -> read-back -> tiny AllGather
barrier -> read M_partner -> fp8 add. Slot addresses come from per-core
int32 offset inputs loaded into SP registers (dge scalar_dynamic_offset);
kernel() runs one warmup execution because the first post-load execution
races DGE descriptor generation against the register loads.

Channel softmax: apply psum = (M/64) q = 16*att_raw, exp via activation
(scale=1/16, bias=-8ln2), ones-matmul denominator, normalize by
256/denom broadcast; proj descales by 1/(16*256) inside its gelu.
LayerNorm runs in row space off the fc_in PSUM (bn_stats); h^T comes
from bf16 DMA transposes + fp8 convert.
"""

import numpy as np
import ml_dtypes

import concourse.bass as bass
import concourse.mybir as mybir
import concourse.tile as tile
from concourse import bacc
from concourse.bass_utils import run_bass_kernel_spmd  # noqa: F401

N_CORES = 8
PAIRS = [[0, 1], [2, 3], [4, 5], [6, 7]]
B, S, C = 4, 2048, 1024
H = 4
S_OWN = S // 2
KT = C // 128          # 8 contraction tiles of 128
NT_OWN = S_OWN // 128  # 8 own t tiles

F32 = mybir.dt.float32
BF16 = mybir.dt.bfloat16
F8 = mybir.dt.float8e4
I32 = mybir.dt.int32
GELU = mybir.ActivationFunctionType.Gelu
EXP = mybir.ActivationFunctionType.Exp
SQRT = mybir.ActivationFunctionType.Sqrt
IDENT = mybir.ActivationFunctionType.Identity
SUB = mybir.AluOpType.subtract
MULT = mybir.AluOpType.mult
BYPASS = mybir.AluOpType.bypass
DR = mybir.MatmulPerfMode.DoubleRow

NP8 = ml_dtypes.float8_e4m3fn

WS = 16.0             # host weight scale
MSC = 64.0            # M stored as M/MSC
APS = C / MSC         # apply psum = APS * att_raw = 16*att
EXP_SHIFT = 8.0       # exp output scaled 2^-8
NORM_SCALE = 256.0    # normalized att stored x256
M_ELEMS = 128 * KT * C  # one M half (1 MB fp8)


def _mm_halves(nc, ps, lhsT_of, rhs_of, n_k, extra=None):
    """Accumulate a [128, 1024] psum tile in two 512-col bank halves with
    DoubleRow fp8 matmuls. lhsT_of(kk) -> [128,2,128]; rhs_of(kk, sl) ->
    [128,2,512]. extra(sl) appends a bias matmul closing the group."""
    for half in range(2):
        sl = slice(half * 512, (half + 1) * 512)
        for kk in range(0, n_k, 2):
            nc.tensor.matmul(ps[:, sl], lhsT_of(kk), rhs_of(kk, sl),
                             start=(kk == 0),
                             stop=(extra is None and kk == n_k - 2),
                             perf_mode=DR)
        if extra is not None:
            extra(sl)


def build(n_heads: int = H, with_bias: bool = True, with_ln_affine: bool = True) -> bacc.Bacc:
    nc = bacc.Bacc(num_devices=N_CORES, name="attn")

    x_t = nc.dram_tensor("x_t", [128, KT, S_OWN], F8, kind="ExternalInput")
    fcw = nc.dram_tensor("fc_in_wT", [128, KT, C], F8, kind="ExternalInput")
    fcb = nc.dram_tensor("fc_in_b_row", [1, C], F8, kind="ExternalInput")
    lng = nc.dram_tensor("ln_g_row", [1, C], F32, kind="ExternalInput")
    lnb = nc.dram_tensor("ln_b_row", [1, C], F32, kind="ExternalInput")
    wq = nc.dram_tensor("wq_t", [H, 128, KT, C], F8, kind="ExternalInput")
    wk = nc.dram_tensor("wk_t", [H, 128, KT, C], F8, kind="ExternalInput")
    wv = nc.dram_tensor("wv_t", [H, 128, KT, C], F8, kind="ExternalInput")
    wp = nc.dram_tensor("wp_t", [H, 128, KT, C], F8, kind="ExternalInput")
    qb = nc.dram_tensor("q_b_col", [H, 128, KT], F32, kind="ExternalInput")
    kb = nc.dram_tensor("k_b_row", [H, 1, C], F8, kind="ExternalInput")
    vb = nc.dram_tensor("v_b_row", [H, 1, C], F8, kind="ExternalInput")
    pb = nc.dram_tensor("proj_b_col", [H, 128, KT], F32, kind="ExternalInput")
    fow = nc.dram_tensor("fc_out_wT", [128, KT, C], F8, kind="ExternalInput")
    fob = nc.dram_tensor("fc_out_b_row", [1, C], F8, kind="ExternalInput")
    offs = nc.dram_tensor("offs", [1, 2], I32, kind="ExternalInput")
    can_in = nc.dram_tensor("can_in", [1, 64], F8, kind="ExternalInput")
    pe = nc.dram_tensor("pe", [S_OWN, C], F32, kind="ExternalInput")
    out = nc.dram_tensor("out", [S_OWN, C], F32, kind="ExternalOutput")
    canary_out = nc.dram_tensor("canary_out", [H, 64], F8, kind="ExternalOutput")

    ROWE = KT * C + 64  # row stride: M payload + canary pad
    hsh = [nc.dram_tensor(f"hsh{i}", [2, 128, ROWE], F8,
                          kind="Internal", addr_space="Shared")
           for i in range(n_heads)]
    bar_in = nc.dram_tensor("bar_in", [1, 3], F8, kind="Internal")
    bar_out = [nc.dram_tensor(f"bar_out{i}", [2, 3], F8, kind="Internal")
               for i in range(n_heads)]

    with tile.TileContext(nc) as tc:
        with (
            tc.tile_pool(name="pers", bufs=1) as pers,
            tc.tile_pool(name="hpool", bufs=2) as hpool,
            tc.tile_pool(name="wpool", bufs=2) as wpool,
            tc.tile_pool(name="psA", bufs=3, space="PSUM") as psA,
            tc.tile_pool(name="psS", bufs=1, space="PSUM") as psS,
            tc.tile_pool(name="small", bufs=2) as small,
            tc.tile_pool(name="act", bufs=3) as actp,
        ):
            ones_col = pers.tile([128, 1], F8)
            nc.vector.memset(ones_col[:], 1.0)
            ones_row8 = pers.tile([1, 128], F8)
            nc.vector.memset(ones_row8[:], 1.0)
            ones_rowb = pers.tile([1, 128], BF16)
            nc.vector.memset(ones_rowb[:], 1.0)
            eps_t = pers.tile([128, 1], F32)
            nc.vector.memset(eps_t[:], 1e-5)
            expb = pers.tile([128, 1], F32)
            nc.vector.memset(expb[:], -float(EXP_SHIFT) * float(np.log(2.0)))
            pred = pers.tile([128, KT, S_OWN], F8, name="pred")

            off_sb = pers.tile([1, 2], I32)
            nc.sync.dma_start(off_sb[:], offs[:])
            r_w = nc.sync.alloc_register("r_w")
            r_r = nc.sync.alloc_register("r_r")
            nc.sync.reg_load(r_w, off_sb[0:1, 0:1])
            nc.sync.reg_load(r_r, off_sb[0:1, 1:2])
            r_wc = nc.sync.alloc_register("r_wc")
            r_rc = nc.sync.alloc_register("r_rc")
            r_w2 = nc.sync.alloc_register("r_w2")
            nc.sync.reg_add(r_wc, r_w, KT * C)
            nc.sync.reg_add(r_rc, r_r, KT * C)
            nc.sync.reg_add(r_w2, r_w, KT * C // 2)
            can_sb = pers.tile([1, 64], F8)
            nc.sync.dma_start(can_sb[:], can_in[:])

            # ================= fc_in + LayerNorm (own rows only) ============
            h_own = hpool.tile([128, KT, S_OWN], F8, tag="hT", name="hT0")
            with tc.tile_pool(name="s0", bufs=1) as s0:
                x_sb = s0.tile([128, KT, S_OWN], F8)
                nc.sync.dma_start(x_sb[:], x_t[:])
                fcw_sb = s0.tile([128, KT, C], F8)
                nc.sync.dma_start(fcw_sb[:], fcw[:])
                if with_bias:
                    fcb_sb = s0.tile([1, C], F8)
                    nc.sync.dma_start(fcb_sb[:], fcb[:])
                if with_ln_affine:
                    g_bc = s0.tile([128, C], F32)
                    nc.sync.dma_start(g_bc[:], bass.AP(tensor=lng, offset=0,
                                                       ap=[[0, 128], [1, C]]))
                    b_bc = s0.tile([128, C], F32)
                    nc.sync.dma_start(b_bc[:], bass.AP(tensor=lnb, offset=0,
                                                       ap=[[0, 128], [1, C]]))
                hTbf = s0.tile([128, KT, S_OWN], BF16)
                for ss in range(NT_OWN):
                    ps = psA.tile([128, C], F32, tag="mmA")
                    _mm_halves(
                        nc, ps,
                        lambda kk, ss=ss: x_sb[:, kk:kk + 2, ss * 128:(ss + 1) * 128],
                        lambda kk, sl: fcw_sb[:, kk:kk + 2, sl], KT,
                        extra=(lambda sl: nc.tensor.matmul(
                            ps[:, sl], ones_row8[:], fcb_sb[0:1, sl],
                            start=False, stop=True)) if with_bias else None)
                    stats = small.tile([128, 2, 6], F32, tag="bnst")
                    nc.vector.bn_stats(stats[:, 0, :], ps[:, 0:512])
                    nc.vector.bn_stats(stats[:, 1, :], ps[:, 512:1024])
                    mv = small.tile([128, 2], F32, tag="mv")
                    nc.vector.bn_aggr(mv[:], stats[:])
                    rstd = small.tile([128, 1], F32, tag="rstd")
                    nc.scalar.activation(rstd[:], mv[:, 1:2], SQRT, bias=eps_t[:], scale=1.0)
                    nc.vector.reciprocal(rstd[:], rstd[:])
                    hnb = s0.tile([128, C], BF16, tag="hnb", bufs=2)
                    if with_ln_affine:
                        hn = s0.tile([128, C], F32, tag="hn", bufs=2)
                        nc.vector.tensor_scalar(hn[:], ps[:], mv[:, 0:1], rstd[:],
                                                op0=SUB, op1=MULT)
                        nc.vector.tensor_mul(hn[:], hn[:], g_bc[:])
                        nc.vector.tensor_add(hnb[:], hn[:], b_bc[:])
                    else:
                        nmu_rs = small.tile([128, 1], F32, tag="nmurs")
                        nc.vector.tensor_scalar(nmu_rs[:], mv[:, 0:1], rstd[:], -1.0,
                                                op0=MULT, op1=MULT)
                        nc.scalar.activation(hnb[:], ps[:], IDENT,
                                             bias=nmu_rs[:], scale=rstd[:])
                    for cc in range(KT):
                        eng = nc.sync if cc % 2 == 0 else nc.scalar
                        eng.dma_start(hTbf[:, cc, ss * 128:(ss + 1) * 128],
                                      hnb[:, cc * 128:(cc + 1) * 128], transpose=True)
                        nc.vector.tensor_copy(h_own[:, cc, ss * 128:(ss + 1) * 128],
                                              hTbf[:, cc, ss * 128:(ss + 1) * 128])

            # ================= heads =================
            for i in range(n_heads):
                wi = i % H
                with tc.tile_pool(name=f"hd{i}", bufs=1) as hp:
                    weng = nc.scalar if i == 0 else nc.sync
                    wq_sb = wpool.tile([128, KT, C], F8, tag="wq")
                    weng.dma_start(wq_sb[:], wq[wi])
                    wk_sb = wpool.tile([128, KT, C], F8, tag="wk")
                    weng.dma_start(wk_sb[:], wk[wi])
                    wv_sb = wpool.tile([128, KT, C], F8, tag="wv")
                    weng.dma_start(wv_sb[:], wv[wi])
                    wp_sb = wpool.tile([128, KT, C], F8, tag="wp")
                    weng.dma_start(wp_sb[:], wp[wi])
                    qb_sb = small.tile([128, KT], F32, tag="qb")
                    nc.sync.dma_start(qb_sb[:], qb[wi])
                    pb_sb = small.tile([128, KT], F32, tag="pb")
                    nc.sync.dma_start(pb_sb[:], pb[wi])
                    if with_bias:
                        kb_sb = small.tile([1, C], F8, tag="kb")
                        nc.sync.dma_start(kb_sb[:], kb[wi])
                        vb_sb = small.tile([1, C], F8, tag="vb")
                        nc.sync.dma_start(vb_sb[:], vb[wi])

                    q_sb = hp.tile([128, KT, S_OWN], F8, name="q_sb")
                    k_sb = hp.tile([128, NT_OWN, C], F8, name="k_sb")
                    v_sb = hp.tile([128, NT_OWN, C], F8, name="v_sb")
                    m_sb = hp.tile([128, KT, C], F8, name="m_sb")
                    mp_sb = hp.tile([128, KT, C], F8, name="mp_sb")
                    attsm = hp.tile([128, KT, S_OWN], F8, name="attsm")

                    # ---- k, v [t, c] (h-stationary) over own rows
                    kv_list = ((k_sb, wk_sb, kb_sb if with_bias else None),
                               (v_sb, wv_sb, vb_sb if with_bias else None))
                    for dst, wmat, bias_sb in kv_list:
                        for tt in range(NT_OWN):
                            ps = psA.tile([128, C], F32, tag="mmA")
                            _mm_halves(
                                nc, ps,
                                lambda kk, tt=tt: h_own[:, kk:kk + 2, tt * 128:(tt + 1) * 128],
                                lambda kk, sl, wmat=wmat: wmat[:, kk:kk + 2, sl], KT,
                                extra=(lambda sl, b=bias_sb: nc.tensor.matmul(
                                    ps[:, sl], ones_row8[:], b[0:1, sl],
                                    start=False, stop=True)) if with_bias else None)
                            nc.scalar.activation(dst[:, tt, :], ps[:], GELU, scale=1.0 / WS)

                    # ---- M_own = v_own^T k_own (x 1/MSC), [c, cq]
                    for co in range(KT):
                        ps = psA.tile([128, C], F32, tag="mmA")
                        _mm_halves(
                            nc, ps,
                            lambda tt, co=co: v_sb[:, tt:tt + 2, co * 128:(co + 1) * 128],
                            lambda tt, sl: k_sb[:, tt:tt + 2, sl], NT_OWN)
                        nc.vector.tensor_scalar_mul(m_sb[:, co, :], ps[:], 1.0 / MSC)

                    # act-table preload: pull the Exp table load off the
                    # post-barrier critical path
                    dummy = small.tile([1, 1], F8, tag="dumm")
                    nc.scalar.activation(dummy[:], can_sb[0:1, 0:1], EXP)

                    # ---- exchange M_own through pair-shared HBM (canary-witnessed)
                    wap = bass.AP(tensor=hsh[i], offset=r_w,
                                  ap=[[ROWE, 128], [1, KT * C // 2]],
                                  dep_tracking_offset=0)
                    nc.sync.dma_start(wap, m_sb[:, 0:KT // 2, :])
                    wap2 = bass.AP(tensor=hsh[i], offset=r_w2,
                                   ap=[[ROWE, 128], [1, KT * C // 2]],
                                   dep_tracking_offset=KT * C // 2)
                    nc.sync.dma_start(wap2, m_sb[:, KT // 2:KT, :])
                    wcap = bass.AP(tensor=hsh[i], offset=r_wc, ap=[[64, 1], [1, 64]],
                                   dep_tracking_offset=KT * C)
                    nc.sync.dma_start(wcap, can_sb[:])
                    rb = hp.tile([1, 3], F8, name="rb")
                    rbap0 = bass.AP(tensor=hsh[i], offset=r_w, ap=[[1, 1], [1, 1]],
                                    dep_tracking_offset=0)
                    nc.sync.dma_start(rb[0:1, 0:1], rbap0)
                    rbap1 = bass.AP(tensor=hsh[i], offset=r_w2, ap=[[1, 1], [1, 1]],
                                    dep_tracking_offset=KT * C // 2)
                    nc.sync.dma_start(rb[0:1, 1:2], rbap1)
                    rbcap = bass.AP(tensor=hsh[i], offset=r_wc, ap=[[1, 1], [1, 1]],
                                    dep_tracking_offset=KT * C)
                    nc.sync.dma_start(rb[0:1, 2:3], rbcap)
                    nc.sync.dma_start(bar_in[:], rb[:])
                    nc.gpsimd.collective_compute(
                        "AllGather", BYPASS, replica_groups=PAIRS,
                        ins=[bar_in[:].opt()], outs=[bar_out[i][:].opt()])
                    # ---- q [co, s] (w-stationary)
                    for co in range(KT):
                        ps = psA.tile([128, S_OWN], F32, tag="mmA")
                        _mm_halves(
                            nc, ps,
                            lambda kk, co=co: wq_sb[:, kk:kk + 2, co * 128:(co + 1) * 128],
                            lambda kk, sl: h_own[:, kk:kk + 2, sl], KT)
                        nc.scalar.activation(q_sb[:, co, :], ps[:], GELU,
                                             bias=qb_sb[:, co:co + 1], scale=1.0 / WS)

                    bar_sb = hp.tile([2, 3], F8, name="bar_sb")
                    nc.sync.dma_start(bar_sb[:], bar_out[i][:])
                    rap = bass.AP(tensor=hsh[i], offset=r_r,
                                  ap=[[ROWE, 128], [1, KT * C]],
                                  dep_tracking_offset=M_ELEMS)
                    nc.sync.dma_start(mp_sb[:], rap)
                    rcap = bass.AP(tensor=hsh[i], offset=r_rc, ap=[[64, 1], [1, 64]],
                                   dep_tracking_offset=M_ELEMS + KT * C)
                    can_rd = hp.tile([1, 64], F8, name="can_rd")
                    nc.sync.dma_start(can_rd[:], rcap)
                    nc.sync.dma_start(canary_out[wi:wi + 1, :], can_rd[:])

                    # ---- apply + exp. co 0/1 fold M_partner into their psum
                    # (start immediately); the DVE add producing msum for the
                    # remaining co runs concurrently on DVE.
                    msum = hp.tile([128, KT, C], F8, name="msum")
                    for cc in range(KT):
                        nc.vector.tensor_add(msum[:, cc, :], m_sb[:, cc, :], mp_sb[:, cc, :])
                    for co in range(KT):
                        ps = psA.tile([128, S_OWN], F32, tag="mmA")
                        if co < 2:
                            for half in range(2):
                                sl = slice(half * 512, (half + 1) * 512)
                                for src_i, msrc in enumerate((m_sb, mp_sb)):
                                    for cc in range(0, KT, 2):
                                        nc.tensor.matmul(
                                            ps[:, sl],
                                            msrc[:, cc:cc + 2, co * 128:(co + 1) * 128],
                                            q_sb[:, cc:cc + 2, sl],
                                            start=(src_i == 0 and cc == 0),
                                            stop=(src_i == 1 and cc == KT - 2),
                                            perf_mode=DR)
                        else:
                            _mm_halves(
                                nc, ps,
                                lambda cc, co=co: msum[:, cc:cc + 2, co * 128:(co + 1) * 128],
                                lambda cc, sl: q_sb[:, cc:cc + 2, sl], KT)
                        nc.scalar.activation(attsm[:, co, :], ps[:], EXP,
                                             bias=expb[:], scale=1.0 / APS)

                    # act-table preload for proj's Gelu
                    dummy2 = small.tile([1, 1], F8, tag="dumm")
                    nc.scalar.activation(dummy2[:], can_sb[0:1, 0:1], GELU)

                    # ---- denominator + normalize (x NORM_SCALE)
                    for half in range(2):
                        sl = slice(half * 512, (half + 1) * 512)
                        dn = psS.tile([1, 512], F32, tag="dn")
                        for cc in range(KT):
                            nc.tensor.matmul(dn[:], ones_col[:], attsm[:, cc, sl],
                                             start=(cc == 0), stop=(cc == KT - 1))
                        rr = small.tile([1, 512], F32, tag="rr")
                        nc.vector.reciprocal(rr[:], dn[:])
                        rrb = small.tile([1, 512], BF16, tag="rrb")
                        nc.vector.tensor_scalar_mul(rrb[:], rr[:], NORM_SCALE)
                        bc = psS.tile([128, 512], F32, tag="bc")
                        nc.tensor.matmul(bc[:], ones_rowb[:], rrb[:], start=True, stop=True)
                        for cc in range(KT):
                            nc.vector.tensor_mul(attsm[:, cc, sl], attsm[:, cc, sl], bc[:])

                    # ---- proj + gelu (+ residual with head-0 output)
                    h_new = hpool.tile([128, KT, S_OWN], F8, tag="hT", name=f"hT{i + 1}")
                    for co in range(KT):
                        ps = psA.tile([128, S_OWN], F32, tag="mmA")
                        _mm_halves(
                            nc, ps,
                            lambda cc, co=co: wp_sb[:, cc:cc + 2, co * 128:(co + 1) * 128],
                            lambda cc, sl: attsm[:, cc:cc + 2, sl], KT)
                        if i == 0:
                            nc.scalar.activation(h_new[:, co, :], ps[:], GELU,
                                                 bias=pb_sb[:, co:co + 1],
                                                 scale=1.0 / (WS * NORM_SCALE))
                            nc.vector.tensor_copy(pred[:, co, :], h_new[:, co, :])
                        else:
                            gt = actp.tile([128, S_OWN], F8, tag="gt")
                            nc.scalar.activation(gt[:], ps[:], GELU,
                                                 bias=pb_sb[:, co:co + 1],
                                                 scale=1.0 / (WS * NORM_SCALE))
                            nc.vector.tensor_add(h_new[:, co, :], gt[:], pred[:, co, :])
                    h_own = h_new

            # ================= fc_out + pose encoding =================
            with tc.tile_pool(name="fo", bufs=1) as fo, \
                 tc.tile_pool(name="fo2", bufs=2) as fo2:
                fow_sb = fo.tile([128, KT, C], F8)
                nc.scalar.dma_start(fow_sb[:], fow[:])
                if with_bias:
                    fob_sb = fo.tile([1, C], F8)
                    nc.sync.dma_start(fob_sb[:], fob[:])
                for ss in range(NT_OWN):
                    ps = psA.tile([128, C], F32, tag="mmA")
                    _mm_halves(
                        nc, ps,
                        lambda kk, ss=ss: h_own[:, kk:kk + 2, ss * 128:(ss + 1) * 128],
                        lambda kk, sl: fow_sb[:, kk:kk + 2, sl], KT,
                        extra=(lambda sl: nc.tensor.matmul(
                            ps[:, sl], ones_row8[:], fob_sb[0:1, sl],
                            start=False, stop=True)) if with_bias else None)
                    obf = fo2.tile([128, C], F32, tag="obf")
                    nc.scalar.activation(obf[:], ps[:], IDENT, scale=1.0 / WS)
                    pe_sb = fo2.tile([128, C], F32, tag="pe")
                    nc.scalar.dma_start(pe_sb[:], pe[ss * 128:(ss + 1) * 128, :])
                    o_sb = fo2.tile([128, C], F32, tag="osb")
                    nc.vector.tensor_add(o_sb[:], obf[:], pe_sb[:])
                    nc.sync.dma_start(out[ss * 128:(ss + 1) * 128, :], o_sb[:])

    nc.compile()
    return nc


def build_null() -> bacc.Bacc:
    """Same I/O signature, ~no compute: measures the dispatch floor."""
    nc = bacc.Bacc(num_devices=N_CORES, name="attn_null")
    nc.dram_tensor("x_t", [128, KT, S_OWN], F8, kind="ExternalInput")
    nc.dram_tensor("fc_in_wT", [128, KT, C], F8, kind="ExternalInput")
    nc.dram_tensor("fc_in_b_row", [1, C], F8, kind="ExternalInput")
    nc.dram_tensor("ln_g_row", [1, C], F32, kind="ExternalInput")
    nc.dram_tensor("ln_b_row", [1, C], F32, kind="ExternalInput")
    nc.dram_tensor("wq_t", [H, 128, KT, C], F8, kind="ExternalInput")
    nc.dram_tensor("wk_t", [H, 128, KT, C], F8, kind="ExternalInput")
    nc.dram_tensor("wv_t", [H, 128, KT, C], F8, kind="ExternalInput")
    nc.dram_tensor("wp_t", [H, 128, KT, C], F8, kind="ExternalInput")
    nc.dram_tensor("q_b_col", [H, 128, KT], F32, kind="ExternalInput")
    nc.dram_tensor("k_b_row", [H, 1, C], F8, kind="ExternalInput")
    nc.dram_tensor("v_b_row", [H, 1, C], F8, kind="ExternalInput")
    nc.dram_tensor("proj_b_col", [H, 128, KT], F32, kind="ExternalInput")
    nc.dram_tensor("fc_out_wT", [128, KT, C], F8, kind="ExternalInput")
    nc.dram_tensor("fc_out_b_row", [1, C], F8, kind="ExternalInput")
    nc.dram_tensor("offs", [1, 2], I32, kind="ExternalInput")
    nc.dram_tensor("can_in", [1, 64], F8, kind="ExternalInput")
    pe = nc.dram_tensor("pe", [S_OWN, C], F32, kind="ExternalInput")
    out = nc.dram_tensor("out", [S_OWN, C], F32, kind="ExternalOutput")
    nc.dram_tensor("canary_out", [H, 64], F8, kind="ExternalOutput")
    with tile.TileContext(nc) as tc:
        with tc.tile_pool(name="p", bufs=2) as p:
            for ss in range(NT_OWN):
                t = p.tile([128, C], F32, tag="t")
                nc.sync.dma_start(t[:], pe[ss * 128:(ss + 1) * 128, :])
                nc.sync.dma_start(out[ss * 128:(ss + 1) * 128, :], t[:])
    nc.compile()
    return nc


def _pose_enc_np(s, f):
    pos = np.arange(s, dtype=np.float32)[:, None]
    div = (1.0 / (1000.0 ** (2.0 * np.arange(f, dtype=np.float32) / np.float32(f))))[None, :]
    p = np.zeros((s, f), np.float32)
    p[0::2, :] = np.sin(pos[0::2] * div)
    p[1::2, :] = np.cos(pos[1::2] * div)
    return p


def _f8(a, scale=1.0):
    return np.ascontiguousarray((np.asarray(a, np.float32) * scale).astype(NP8))


def _tile_kt(mat):
    """[C_in, N] -> [128, KT, N] (c_in = kt*128 + partition)."""
    cin, n = mat.shape
    return np.ascontiguousarray(mat.reshape(KT, 128, n).transpose(1, 0, 2))


def prepare_in_maps(x, fc_in_w, fc_in_b, ln_g, ln_b, qkv_w, qkv_b, proj_w, proj_b,
                    fc_out_w, fc_out_b):
    x = np.asarray(x, np.float32)
    qkv_w = np.asarray(qkv_w, np.float32)
    qkv_b = np.asarray(qkv_b, np.float32)
    proj_w = np.asarray(proj_w, np.float32)
    proj_b = np.asarray(proj_b, np.float32)

    shared = {
        "fc_in_wT": _tile_kt(_f8(np.asarray(fc_in_w, np.float32).T, WS)),
        "fc_in_b_row": _f8(np.asarray(fc_in_b)[None, :], WS),
        "ln_g_row": np.ascontiguousarray(np.asarray(ln_g, np.float32)[None, :]),
        "ln_b_row": np.ascontiguousarray(np.asarray(ln_b, np.float32)[None, :]),
        "wq_t": np.stack([_tile_kt(_f8(qkv_w[i, 0:C, :].T, WS)) for i in range(H)]),
        "wk_t": np.stack([_tile_kt(_f8(qkv_w[i, C:2 * C, :].T, WS)) for i in range(H)]),
        "wv_t": np.stack([_tile_kt(_f8(qkv_w[i, 2 * C:, :].T, WS)) for i in range(H)]),
        "wp_t": np.stack([_tile_kt(_f8(proj_w[i].T, WS)) for i in range(H)]),
        "q_b_col": np.ascontiguousarray(
            qkv_b[:, 0:C].reshape(H, KT, 128).transpose(0, 2, 1)),
        "k_b_row": _f8(qkv_b[:, C:2 * C][:, None, :], WS),
        "v_b_row": _f8(qkv_b[:, 2 * C:][:, None, :], WS),
        "proj_b_col": np.ascontiguousarray(
            proj_b.reshape(H, KT, 128).transpose(0, 2, 1)),
        "fc_out_wT": _tile_kt(_f8(np.asarray(fc_out_w, np.float32).T, WS)),
        "fc_out_b_row": _f8(np.asarray(fc_out_b)[None, :], WS),
    }
    pe_full = _pose_enc_np(S, C)
    in_maps = []
    for core in range(N_CORES):
        b, half = divmod(core, 2)
        own = x[b, half * S_OWN:(half + 1) * S_OWN, :].T  # [C, S_OWN]
        m = dict(shared)
        m["x_t"] = _tile_kt(_f8(own))
        m["pe"] = np.ascontiguousarray(pe_full[half * S_OWN:(half + 1) * S_OWN, :])
        slot_elems = 128 * (KT * C + 64)
        m["offs"] = np.array([[half * slot_elems, (1 - half) * slot_elems]], np.int32)
        m["can_in"] = np.full((1, 64), core + 1, NP8)
        in_maps.append(m)
    return in_maps


_NC_CACHE = {}


def get_nc(n_heads=H, with_bias=True, with_ln_affine=True):
    key = (n_heads, with_bias, with_ln_affine)
    if key not in _NC_CACHE:
        _NC_CACHE[key] = build(n_heads, with_bias, with_ln_affine)
    return _NC_CACHE[key]


_EXEC_CACHE = {}


def _get_executable(nc):
    """One jitted collectives executable per process (loading a second one
    hangs the axon worker); reused across kernel() calls."""
    key = id(nc)
    if key in _EXEC_CACHE:
        return _EXEC_CACHE[key]
    import jax
    from jax.sharding import Mesh, PartitionSpec, NamedSharding
    from jax.experimental.shard_map import shard_map
    from concourse import bass2jax
    import concourse.mybir as mybir_

    bass2jax.install_neuronx_cc_hook()
    partition_name = nc.partition_id_tensor.name if nc.partition_id_tensor else None
    in_names, out_names, out_avals, zero_outs = [], [], [], []
    for alloc in nc.m.functions[0].allocations:
        if not isinstance(alloc, mybir_.MemoryLocationSet):
            continue
        name = alloc.memorylocations[0].name
        if alloc.kind == "ExternalInput":
            if name != partition_name:
                in_names.append(name)
        elif alloc.kind == "ExternalOutput":
            out_names.append(name)
            shape = tuple(alloc.tensor_shape)
            dtype = mybir_.dt.np(alloc.dtype)
            out_avals.append(jax.core.ShapedArray(shape, dtype))
            zero_outs.append(np.zeros(shape, dtype))
    n_params = len(in_names)
    n_outs = len(out_avals)
    all_in = in_names + out_names + ([partition_name] if partition_name else [])
    donate = tuple(range(n_params, n_params + n_outs))

    def _body(*args):
        operands = list(args)
        if partition_name is not None:
            operands.append(bass2jax.partition_id_tensor())
        return tuple(bass2jax._bass_exec_p.bind(
            *operands, out_avals=tuple(out_avals), in_names=tuple(all_in),
            out_names=tuple(out_names), lowering_input_output_aliases=(),
            sim_require_finite=True, sim_require_nnan=True, nc=nc))

    devices = jax.devices()[:N_CORES]
    mesh = Mesh(np.asarray(devices), ("core",))
    sharded = jax.jit(
        shard_map(_body, mesh=mesh,
                  in_specs=(PartitionSpec("core"),) * (n_params + n_outs),
                  out_specs=(PartitionSpec("core"),) * len(out_names),
                  check_rep=False),
        donate_argnums=donate, keep_unused=True)
    sh = NamedSharding(mesh, PartitionSpec("core"))
    entry = (sharded, sh, in_names[:n_params], out_names, out_avals, zero_outs)
    _EXEC_CACHE[key] = entry
    return entry


def flags_for(inputs):
    with_bias = not (np.all(np.asarray(inputs["fc_in_b"]) == 0)
                     and np.all(np.asarray(inputs["qkv_b"]) == 0)
                     and np.all(np.asarray(inputs["proj_b"]) == 0)
                     and np.all(np.asarray(inputs["fc_out_b"]) == 0))
    with_ln = not (np.all(np.asarray(inputs["ln_g"]) == 1)
                   and np.all(np.asarray(inputs["ln_b"]) == 0))
    return with_bias, with_ln


_WARMED = set()


def canaries_ok(out_arrs, out_names, out_avals):
    """True iff every core read its partner's canary in every head: proves the
    dynamic-offset registers were correct for that execution's exchange."""
    ci = out_names.index("canary_out")
    pc = np.asarray(out_arrs[ci]).reshape(N_CORES, *out_avals[ci].shape)
    vals = pc.astype(np.float32)
    for core in range(N_CORES):
        if not np.all(vals[core] == float((core ^ 1) + 1)):
            return False
    return True


def kernel(**inputs) -> np.ndarray:
    with_bias, with_ln = flags_for(inputs)
    nc = get_nc(H, with_bias, with_ln)
    in_maps = prepare_in_maps(**inputs)
    import jax
    sharded, sh, in_names, out_names, out_avals, zero_outs = _get_executable(nc)
    concat_in = [jax.device_put(
        np.concatenate([np.asarray(in_maps[c][nm]) for c in range(N_CORES)], axis=0), sh)
        for nm in in_names]

    def one_call():
        concat_zeros = [jax.device_put(
            np.zeros((N_CORES * z.shape[0], *z.shape[1:]), z.dtype), sh)
            for z in zero_outs]
        out_arrs = sharded(*concat_in, *concat_zeros)
        jax.block_until_ready(out_arrs)
        return out_arrs

    # The first executions after NEFF load race DGE descriptor generation
    # against the dynamic-offset register loads; registers persist across
    # executions, so retry until the canaries prove the exchange addressed
    # the right slots (typically clean from the 2nd execution).
    if id(nc) not in _WARMED:
        one_call()
        _WARMED.add(id(nc))
    for _attempt in range(8):
        out_arrs = one_call()
        oi_ = out_names.index("out")
        pc_ = np.asarray(out_arrs[oi_])
        if canaries_ok(out_arrs, out_names, out_avals) and not np.isnan(pc_).any():
            break
    oi = out_names.index("out")
    per_core = np.asarray(out_arrs[oi]).reshape(N_CORES, *out_avals[oi].shape)
    out_full = np.empty((B, S, C), np.float32)
    for core in range(N_CORES):
        b, half = divmod(core, 2)
        out_full[b, half * S_OWN:(half + 1) * S_OWN, :] = per_core[core]
    return out_full



# revision 6
# speedup vs baseline: 1.2695x; 1.2695x over previous
"""Distributed Trainium2 kernel for nn_Attention_18562848653411 (v2).

Reference model: fc_in -> LayerNorm -> 4 sequential "refinement heads"
(qkv matmul + gelu, scores=q@k^T/C, att=scores@v, softmax over channels,
proj + gelu, residual with head-0 output) -> fc_out + PoseEncoding.

Key algebra: softmax comes AFTER att = scores@v, so per head
att^T = (v^T k) q^T / C = M q^T / C with M = v^T k a [C,C] matrix that
is a sum over sequence positions. No S x S scores are ever formed.

Sharding (8 NeuronCores): core c handles batch b=c//2, sequence half
c%2. All weights replicated, fp8e4 (x16 host scale); every big matmul is
a DoubleRow fp8 instruction. Per head each core computes k/v and
M_own = v_own^T k_own over its OWN 1024 rows and exchanges M_own (1 MB
fp8) with its pair partner through pair-shared HBM. The pair barrier
(tiny AllGather) is issued EARLY (right after k/v) so its ~15us fixed
latency overlaps the M matmuls and M writes; per-head canary values
(core+1)*2^head written after the M payload on the same queue witness
that the partner's writes landed before our read -- kernel() retries
until canaries prove a clean exchange.

Softmax over channels runs on transposed tiles att^T[c, s]: exp via
activation (scale=1/16, bias=-8ln2), DoubleRow ones-matmul denominator,
gpsimd partition-broadcast of 256/denom, DVE normalize; proj descales by
1/(16*256) inside its gelu. The apply step accumulates m_own q and
m_partner q directly in PSUM (no pre-add of the M halves). LayerNorm
runs in row space off the fc_in PSUM (bn_stats, Act does only Sqrt so a
single act table serves all of fc_in); h^T comes from PE transposes +
Act Copy (no DMA transposes).
"""

import numpy as np
import ml_dtypes

import concourse.bass as bass
import concourse.mybir as mybir
import concourse.tile as tile
from concourse import bacc
from concourse.bass_utils import run_bass_kernel_spmd  # noqa: F401
from concourse.masks import make_identity

N_CORES = 8
PAIRS = [[0, 1], [2, 3], [4, 5], [6, 7]]
B, S, C = 4, 2048, 1024
H = 4
S_OWN = S // 2
KT = C // 128          # 8 contraction tiles of 128
NT_OWN = S_OWN // 128  # 8 own t tiles
HALF = 512

F32 = mybir.dt.float32
BF16 = mybir.dt.bfloat16
F8 = mybir.dt.float8e4
I32 = mybir.dt.int32
GELU = mybir.ActivationFunctionType.Gelu
EXP = mybir.ActivationFunctionType.Exp
SQRT = mybir.ActivationFunctionType.Sqrt
COPY = mybir.ActivationFunctionType.Copy
SUB = mybir.AluOpType.subtract
MULT = mybir.AluOpType.mult
BYPASS = mybir.AluOpType.bypass
DR = mybir.MatmulPerfMode.DoubleRow

NP8 = ml_dtypes.float8_e4m3fn
NPBF = ml_dtypes.bfloat16

WS = 16.0             # host weight scale
MSC = 64.0            # M stored as M/MSC
APS = C / MSC         # apply psum = APS * att_raw = 16*att
EXP_SHIFT = 8.0       # exp output scaled 2^-8
NORM_SCALE = 256.0    # normalized att stored x256
M_ELEMS = 128 * KT * C  # one M half (1 MB fp8)
GATE_TT = 1  # v tile whose completion launches the pair barrier

PHASE_MARKS = []


def _mark(nc, name):
    PHASE_MARKS.append((name, int(nc.get_next_instruction_name().split("-")[1])))


def _mm_full(nc, ps, lhsT_of, rhs_of, n_k, extra=None):
    """Accumulate a [128, 1024] psum tile in two 512-col bank halves with
    DoubleRow fp8 matmuls. lhsT_of(kk) -> [128,2,128]; rhs_of(kk, sl) ->
    [128,2,512]. extra(sl) appends a bias matmul closing the group."""
    for half in range(2):
        sl = slice(half * HALF, (half + 1) * HALF)
        for kk in range(0, n_k, 2):
            nc.tensor.matmul(ps[:, sl], lhsT_of(kk), rhs_of(kk, sl),
                             start=(kk == 0),
                             stop=(extra is None and kk == n_k - 2),
                             perf_mode=DR)
        if extra is not None:
            extra(sl)


def build(n_heads: int = H, with_bias: bool = True, with_ln_affine: bool = True) -> bacc.Bacc:
    PHASE_MARKS.clear()
    nc = bacc.Bacc(num_devices=N_CORES, name="attn")

    x_t = nc.dram_tensor("x_t", [128, KT, S_OWN], F8, kind="ExternalInput")
    fcw = nc.dram_tensor("fc_in_wT", [128, KT, C], F8, kind="ExternalInput")
    fcb = nc.dram_tensor("fc_in_b_row", [1, C], F8, kind="ExternalInput")
    lng = nc.dram_tensor("ln_g_row", [1, C], F32, kind="ExternalInput")
    lnb = nc.dram_tensor("ln_b_row", [1, C], F32, kind="ExternalInput")
    wq = nc.dram_tensor("wq_t", [H, 128, KT, C], F8, kind="ExternalInput")
    wk = nc.dram_tensor("wk_t", [H, 128, KT, C], F8, kind="ExternalInput")
    wv = nc.dram_tensor("wv_t", [H, 128, KT, C], F8, kind="ExternalInput")
    wp = nc.dram_tensor("wp_t", [H, 128, KT, C], F8, kind="ExternalInput")
    qb = nc.dram_tensor("q_b_col", [H, 128, KT], F32, kind="ExternalInput")
    kb = nc.dram_tensor("k_b_row", [H, 1, C], F8, kind="ExternalInput")
    vb = nc.dram_tensor("v_b_row", [H, 1, C], F8, kind="ExternalInput")
    pb = nc.dram_tensor("proj_b_col", [H, 128, KT], F32, kind="ExternalInput")
    fow = nc.dram_tensor("fc_out_wT", [128, KT, C], F8, kind="ExternalInput")
    fob = nc.dram_tensor("fc_out_b_row", [1, C], F8, kind="ExternalInput")
    offs = nc.dram_tensor("offs", [1, 2], I32, kind="ExternalInput")
    can_in = nc.dram_tensor("can_in", [H, 64], F8, kind="ExternalInput")
    pe = nc.dram_tensor("pe", [S_OWN, C], BF16, kind="ExternalInput")
    out = nc.dram_tensor("out", [S_OWN, C], F32, kind="ExternalOutput")
    canary_out = nc.dram_tensor("canary_out", [H, 64], F8, kind="ExternalOutput")

    ROWE = KT * C + 64  # row stride: M payload + canary pad
    hsh = [nc.dram_tensor(f"hsh{i}", [2, 128, ROWE], F8,
                          kind="Internal", addr_space="Shared")
           for i in range(n_heads)]
    bar_in = nc.dram_tensor("bar_in", [1, 3], F8, kind="Internal")
    bar_out = [nc.dram_tensor(f"bar_out{i}", [2, 3], F8, kind="Internal")
               for i in range(n_heads)]

    with tile.TileContext(nc) as tc:
        with (
            tc.tile_pool(name="pers", bufs=1) as pers,
            tc.tile_pool(name="hpool", bufs=2) as hpool,
            tc.tile_pool(name="wpool", bufs=2) as wpool,
            tc.tile_pool(name="small", bufs=2) as small,
            tc.tile_pool(name="act", bufs=3) as actp,
        ):
            ones_col = pers.tile([128, 1], F8)
            nc.vector.memset(ones_col[:], 1.0)
            ones_row8 = pers.tile([1, 128], F8)
            nc.vector.memset(ones_row8[:], 1.0)
            eps_t = pers.tile([128, 1], F32)
            nc.vector.memset(eps_t[:], 1e-5)
            expb = pers.tile([128, 1], F32)
            nc.vector.memset(expb[:], -float(EXP_SHIFT) * float(np.log(2.0)))
            ident = pers.tile([128, 128], BF16)
            make_identity(nc, ident[:])
            pred = pers.tile([128, KT, S_OWN], F8, name="pred")
            pe_sb = pers.tile([128, NT_OWN, C], BF16, name="pe_sb")

            off_sb = pers.tile([1, 2], I32)
            nc.sync.dma_start(off_sb[:], offs[:])
            r_w = nc.sync.alloc_register("r_w")
            r_r = nc.sync.alloc_register("r_r")
            nc.sync.reg_load(r_w, off_sb[0:1, 0:1])
            nc.sync.reg_load(r_r, off_sb[0:1, 1:2])
            r_wc = nc.sync.alloc_register("r_wc")
            r_rc = nc.sync.alloc_register("r_rc")
            r_w2 = nc.sync.alloc_register("r_w2")
            r_r2 = nc.sync.alloc_register("r_r2")
            nc.sync.reg_add(r_wc, r_w, KT * C)
            nc.sync.reg_add(r_rc, r_r, KT * C)
            nc.sync.reg_add(r_w2, r_w, KT * C // 2)
            nc.sync.reg_add(r_r2, r_r, KT * C // 2)
            can_sb = pers.tile([H, 64], F8)
            nc.sync.dma_start(can_sb[:], can_in[:])

            # head-0 weights prefetch on gpsimd (SWDGE; no HWDGE contention);
            # wk/wv first -- the fc_in loop interleaves head-0 k/v matmuls
            wk_sb = wpool.tile([128, KT, C], F8, tag="wk")
            nc.gpsimd.dma_start(wk_sb[:], wk[0])
            wv_sb = wpool.tile([128, KT, C], F8, tag="wv")
            nc.gpsimd.dma_start(wv_sb[:], wv[0])
            wq_sb = wpool.tile([128, KT, C], F8, tag="wq")
            nc.gpsimd.dma_start(wq_sb[:], wq[0])
            wp_sb = wpool.tile([128, KT, C], F8, tag="wp")
            nc.gpsimd.dma_start(wp_sb[:], wp[0])
            if with_bias:
                kb_sb = small.tile([1, C], F8, tag="kb")
                nc.sync.dma_start(kb_sb[:], kb[0])
                vb_sb = small.tile([1, C], F8, tag="vb")
                nc.sync.dma_start(vb_sb[:], vb[0])
            k0_sb = None  # head-0 k/v tiles, filled by the fc_in loop

            # ================= fc_in + LayerNorm (own rows only) ============
            _mark(nc, "fc_in")
            h_own = hpool.tile([128, KT, S_OWN], F8, tag="hT", name="hT0")
            k0_sb = pers.tile([128, NT_OWN, C], F8, name="k0_sb")
            v0_sb = pers.tile([128, NT_OWN, C], F8, name="v0_sb")
            with (tc.tile_pool(name="s0", bufs=1) as s0,
                  tc.tile_pool(name="s0ps", bufs=2, space="PSUM") as s0ps,
                  tc.tile_pool(name="s0tp", bufs=2, space="PSUM") as s0tp):
                x_sb = s0.tile([128, KT, S_OWN], F8)
                nc.sync.dma_start(x_sb[:], x_t[:])
                fcw_sb = s0.tile([128, KT, C], F8)
                nc.sync.dma_start(fcw_sb[:], fcw[:])
                if with_bias:
                    fcb_sb = s0.tile([1, C], F8)
                    nc.sync.dma_start(fcb_sb[:], fcb[:])
                if with_ln_affine:
                    g_bc = s0.tile([128, C], F32)
                    nc.sync.dma_start(g_bc[:], bass.AP(tensor=lng, offset=0,
                                                       ap=[[0, 128], [1, C]]))
                    b_bc = s0.tile([128, C], F32)
                    nc.sync.dma_start(b_bc[:], bass.AP(tensor=lnb, offset=0,
                                                       ap=[[0, 128], [1, C]]))
                for blk in range(2):
                    # 4 fc_in tiles (Act: sqrt+copy, one table); the PE
                    # transposes trail the mm/LN chain by one tile so the next
                    # matmul never waits on the DVE normalize
                    hnb_q = []

                    def _transp(ss, hnb):
                        for grp in range(2):
                            tp = s0tp.tile([128, 4, 128], BF16, tag="tp")
                            for j in range(4):
                                cc = grp * 4 + j
                                nc.tensor.transpose(
                                    tp[:, j, :], hnb[:, cc * 128:(cc + 1) * 128], ident[:])
                            nc.scalar.activation(
                                h_own[:, grp * 4:(grp + 1) * 4, ss * 128:(ss + 1) * 128],
                                tp[:], COPY)

                    for ss in range(blk * 4, blk * 4 + 4):
                        ps = s0ps.tile([128, C], F32, tag="mmA")
                        _mm_full(
                            nc, ps,
                            lambda kk, ss=ss: x_sb[:, kk:kk + 2, ss * 128:(ss + 1) * 128],
                            lambda kk, sl: fcw_sb[:, kk:kk + 2, sl], KT,
                            extra=(lambda sl: nc.tensor.matmul(
                                ps[:, sl], ones_row8[:], fcb_sb[0:1, sl],
                                start=False, stop=True)) if with_bias else None)
                        stats = small.tile([128, 2, 6], F32, tag="bnst")
                        nc.vector.bn_stats(stats[:, 0, :], ps[:, 0:HALF])
                        nc.vector.bn_stats(stats[:, 1, :], ps[:, HALF:C])
                        mv = small.tile([128, 2], F32, tag="mv")
                        nc.vector.bn_aggr(mv[:], stats[:])
                        rstd = small.tile([128, 1], F32, tag="rstd")
                        nc.scalar.activation(rstd[:], mv[:, 1:2], SQRT, bias=eps_t[:], scale=1.0)
                        nc.vector.reciprocal(rstd[:], rstd[:])
                        hnb = s0.tile([128, C], BF16, tag="hnb", bufs=3)
                        nc.vector.tensor_scalar(hnb[:], ps[:], mv[:, 0:1], rstd[:],
                                                op0=SUB, op1=MULT)
                        if with_ln_affine:
                            hnf = s0.tile([128, C], F32, tag="hnf", bufs=2)
                            nc.vector.tensor_mul(hnf[:], hnb[:], g_bc[:])
                            nc.vector.tensor_add(hnb[:], hnf[:], b_bc[:])
                        hnb_q.append((ss, hnb))
                        if len(hnb_q) > 1:
                            _transp(*hnb_q.pop(0))
                    while hnb_q:
                        _transp(*hnb_q.pop(0))
                    # ... then head-0 k/v for those tiles (Act: gelu) -- block
                    # granularity keeps act-table transitions to one per block
                    for tt in range(blk * 4, blk * 4 + 4):
                        for dst, wmat, bias_sb in ((k0_sb, wk_sb, kb_sb if with_bias else None),
                                                   (v0_sb, wv_sb, vb_sb if with_bias else None)):
                            ps = s0ps.tile([128, C], F32, tag="mmA")
                            _mm_full(
                                nc, ps,
                                lambda kk, tt=tt: h_own[:, kk:kk + 2, tt * 128:(tt + 1) * 128],
                                lambda kk, sl, wmat=wmat: wmat[:, kk:kk + 2, sl], KT,
                                extra=(lambda sl, b=bias_sb: nc.tensor.matmul(
                                    ps[:, sl], ones_row8[:], b[0:1, sl],
                                    start=False, stop=True)) if with_bias else None)
                            nc.scalar.activation(dst[:, tt, :], ps[:], GELU, scale=1.0 / WS)

            # (gelu table load happens at head 0's first kv gelu)

            # ================= heads =================
            with (tc.tile_pool(name="psA", bufs=2, space="PSUM") as psA,
                  tc.tile_pool(name="psH", bufs=2, space="PSUM") as psH,
                  tc.tile_pool(name="psD", bufs=2, space="PSUM") as psD,
                  tc.tile_pool(name="fo2", bufs=2) as fo2):
                for i in range(n_heads):
                    wi = i % H
                    _mark(nc, f"head{i}")
                    with tc.tile_pool(name=f"hd{i}", bufs=1) as hp:
                        if with_bias:
                            if i > 0:
                                kb_sb = small.tile([1, C], F8, tag="kb")
                                nc.sync.dma_start(kb_sb[:], kb[wi])
                                vb_sb = small.tile([1, C], F8, tag="vb")
                                nc.sync.dma_start(vb_sb[:], vb[wi])
                            qb_sb = small.tile([128, KT], F32, tag="qb")
                            nc.sync.dma_start(qb_sb[:], qb[wi])
                            pb_sb = small.tile([128, KT], F32, tag="pb")
                            nc.sync.dma_start(pb_sb[:], pb[wi])

                        q_sb = hp.tile([128, KT, S_OWN], F8, name="q_sb")
                        if i == 0:
                            k_sb, v_sb = k0_sb, v0_sb
                        else:
                            k_sb = hp.tile([128, NT_OWN, C], F8, name="k_sb")
                            v_sb = hp.tile([128, NT_OWN, C], F8, name="v_sb")
                        m_sb = hp.tile([128, KT, C], F8, name="m_sb")
                        mp_sb = hp.tile([128, KT, C], F8, name="mp_sb")
                        attsm = hp.tile([128, KT, S_OWN], F8, name="attsm")

                        # ---- k, v [t, c] (h-stationary) over own rows
                        # (head 0's k/v were interleaved into the fc_in loop)
                        if i > 0:
                            kv_list = ((k_sb, wk_sb, kb_sb if with_bias else None),
                                       (v_sb, wv_sb, vb_sb if with_bias else None))
                            for dst, wmat, bias_sb in kv_list:
                                for tt in range(NT_OWN):
                                    ps = psA.tile([128, C], F32, tag="mmA")
                                    _mm_full(
                                        nc, ps,
                                        lambda kk, tt=tt: h_own[:, kk:kk + 2, tt * 128:(tt + 1) * 128],
                                        lambda kk, sl, wmat=wmat: wmat[:, kk:kk + 2, sl], KT,
                                        extra=(lambda sl, b=bias_sb: nc.tensor.matmul(
                                            ps[:, sl], ones_row8[:], b[0:1, sl],
                                            start=False, stop=True)) if with_bias else None)
                                    nc.scalar.activation(dst[:, tt, :], ps[:], GELU, scale=1.0 / WS)

                        # ---- early pair barrier: gate bar_in on v's last tile
                        # so the ~15us collective overlaps the M matmuls and
                        # M writes (canaries verify the race was won).
                        _mark(nc, f"h{i}_bar")
                        # barrier payload is irrelevant; source it from v tile 3
                        # so the collective launches once v is half done (the
                        # remaining ~15us of barrier covers M compute + writes
                        # on both cores; canaries verify the race was won)
                        nc.sync.dma_start(bar_in[:], v_sb[0:1, GATE_TT, 0:3])
                        nc.gpsimd.collective_compute(
                            "AllGather", BYPASS, replica_groups=PAIRS,
                            ins=[bar_in[:].opt()], outs=[bar_out[i][:].opt()])

                        # next-head weight prefetch (gpsimd queue, after the
                        # collective so transfers run under the barrier)
                        if i + 1 < n_heads:
                            nwi = (i + 1) % H
                            wq_n = wpool.tile([128, KT, C], F8, tag="wq")
                            nc.gpsimd.dma_start(wq_n[:], wq[nwi])
                            wk_n = wpool.tile([128, KT, C], F8, tag="wk")
                            nc.gpsimd.dma_start(wk_n[:], wk[nwi])
                            wv_n = wpool.tile([128, KT, C], F8, tag="wv")
                            nc.gpsimd.dma_start(wv_n[:], wv[nwi])
                            wp_n = wpool.tile([128, KT, C], F8, tag="wp")
                            nc.gpsimd.dma_start(wp_n[:], wp[nwi])
                        if i == 1:
                            nc.gpsimd.dma_start(
                                pe_sb[:], bass.AP(tensor=pe, offset=0,
                                                  ap=[[C, 128], [128 * C, NT_OWN], [1, C]]))
                        if i == 2:
                            fow_sb = pers.tile([128, KT, C], F8, name="fow_sb")
                            nc.gpsimd.dma_start(fow_sb[:], fow[:])

                        # ---- M_own = v_own^T k_own (x 1/MSC), [c, cq]
                        _mark(nc, f"h{i}_M")
                        for co in range(KT):
                            ps = psA.tile([128, C], F32, tag="mmA")
                            _mm_full(
                                nc, ps,
                                lambda tt, co=co: v_sb[:, tt:tt + 2, co * 128:(co + 1) * 128],
                                lambda tt, sl: k_sb[:, tt:tt + 2, sl], NT_OWN)
                            nc.vector.tensor_scalar_mul(m_sb[:, co, :], ps[:], 1.0 / MSC)
                            if co == KT // 2 - 1:
                                wap = bass.AP(tensor=hsh[i], offset=r_w,
                                              ap=[[ROWE, 128], [1, KT * C // 2]],
                                              dep_tracking_offset=0)
                                nc.sync.dma_start(wap, m_sb[:, 0:KT // 2, :])
                        wap2 = bass.AP(tensor=hsh[i], offset=r_w2,
                                       ap=[[ROWE, 128], [1, KT * C // 2]],
                                       dep_tracking_offset=KT * C // 2)
                        nc.sync.dma_start(wap2, m_sb[:, KT // 2:KT, :])
                        wcap = bass.AP(tensor=hsh[i], offset=r_wc, ap=[[64, 1], [1, 64]],
                                       dep_tracking_offset=KT * C)
                        nc.sync.dma_start(wcap, can_sb[wi:wi + 1, :])

                        # ---- q [co, s] (w-stationary) -- fills barrier window
                        _mark(nc, f"h{i}_q")
                        for co in range(KT):
                            ps = psA.tile([128, S_OWN], F32, tag="mmA")
                            _mm_full(
                                nc, ps,
                                lambda kk, co=co: wq_sb[:, kk:kk + 2, co * 128:(co + 1) * 128],
                                lambda kk, sl: h_own[:, kk:kk + 2, sl], KT)
                            if with_bias:
                                nc.scalar.activation(q_sb[:, co, :], ps[:], GELU,
                                                     bias=qb_sb[:, co:co + 1], scale=1.0 / WS)
                            else:
                                nc.scalar.activation(q_sb[:, co, :], ps[:], GELU,
                                                     scale=1.0 / WS)

                        # (exp table load happens at the first apply exp)

                        # ---- barrier done: canary first, then partner M
                        bar_sb = hp.tile([2, 3], F8, name="bar_sb")
                        nc.sync.dma_start(bar_sb[:], bar_out[i][:])
                        rcap = bass.AP(tensor=hsh[i], offset=r_rc, ap=[[64, 1], [1, 64]],
                                       dep_tracking_offset=M_ELEMS + KT * C)
                        can_rd = hp.tile([1, 64], F8, name="can_rd")
                        nc.sync.dma_start(can_rd[:], rcap)
                        nc.sync.dma_start(canary_out[wi:wi + 1, :], can_rd[:])
                        rap0 = bass.AP(tensor=hsh[i], offset=r_r,
                                       ap=[[ROWE, 128], [1, KT * C // 2]],
                                       dep_tracking_offset=M_ELEMS)
                        nc.sync.dma_start(mp_sb[:, 0:KT // 2, :], rap0)
                        rap1 = bass.AP(tensor=hsh[i], offset=r_r2,
                                       ap=[[ROWE, 128], [1, KT * C // 2]],
                                       dep_tracking_offset=M_ELEMS + KT * C // 2)
                        nc.sync.dma_start(mp_sb[:, KT // 2:KT, :], rap1)

                        # ---- apply + exp, per s-half: accumulate m and mp
                        # contributions straight in PSUM (no M pre-add)
                        _mark(nc, f"h{i}_apply")
                        for half in range(2):
                            sl = slice(half * HALF, (half + 1) * HALF)
                            for co in range(KT):
                                ps = psH.tile([128, HALF], F32, tag="mm5")
                                for src_i, msrc in enumerate((m_sb, mp_sb)):
                                    for cc in range(0, KT, 2):
                                        nc.tensor.matmul(
                                            ps[:],
                                            msrc[:, cc:cc + 2, co * 128:(co + 1) * 128],
                                            q_sb[:, cc:cc + 2, sl],
                                            start=(src_i == 0 and cc == 0),
                                            stop=(src_i == 1 and cc == KT - 2),
                                            perf_mode=DR)
                                nc.scalar.activation(attsm[:, co, sl], ps[:], EXP,
                                                     bias=expb[:], scale=1.0 / APS)
                            # denominator (DoubleRow ones) + 256/denom
                            # broadcast, pipelined per s-quarter so proj can
                            # chase the normalize front
                            for qq in range(2):
                                qsl = slice(half * HALF + qq * 256,
                                            half * HALF + (qq + 1) * 256)
                                dn = psD.tile([1, 256], F32, tag="dn")
                                for cc in range(KT):
                                    nc.tensor.matmul(dn[:], ones_col[:],
                                                     attsm[:, cc, qsl],
                                                     start=(cc == 0), stop=(cc == KT - 1))
                                rr = small.tile([1, 256], F32, tag="rr")
                                nc.vector.reciprocal(rr[:], dn[:])
                                bc = small.tile([128, 256], F32, tag="bc")
                                nc.gpsimd.partition_broadcast(bc[:], rr[0:1, :], channels=128)
                                for cc in range(KT):
                                    nc.vector.scalar_tensor_tensor(
                                        attsm[:, cc, qsl], attsm[:, cc, qsl], NORM_SCALE,
                                        bc[:], op0=MULT, op1=MULT)

                        # (gelu table load happens at the first proj gelu)

                        # ---- proj + gelu (+ residual with head-0 output)
                        _mark(nc, f"h{i}_proj")
                        h_new = hpool.tile([128, KT, S_OWN], F8, tag="hT", name=f"hT{i + 1}")
                        for half in range(2):
                            sl = slice(half * HALF, (half + 1) * HALF)
                            for co in range(KT):
                                ps = psH.tile([128, HALF], F32, tag="mm5")
                                for cc in range(0, KT, 2):
                                    nc.tensor.matmul(
                                        ps[:],
                                        wp_sb[:, cc:cc + 2, co * 128:(co + 1) * 128],
                                        attsm[:, cc:cc + 2, sl],
                                        start=(cc == 0), stop=(cc == KT - 2),
                                        perf_mode=DR)
                                bias_kw = (dict(bias=pb_sb[:, co:co + 1])
                                           if with_bias else {})
                                if i == 0:
                                    nc.scalar.activation(h_new[:, co, sl], ps[:], GELU,
                                                         scale=1.0 / (WS * NORM_SCALE),
                                                         **bias_kw)
                                    nc.vector.tensor_copy(pred[:, co, sl], h_new[:, co, sl])
                                else:
                                    gt = actp.tile([128, HALF], F8, tag="gt")
                                    nc.scalar.activation(gt[:], ps[:], GELU,
                                                         scale=1.0 / (WS * NORM_SCALE),
                                                         **bias_kw)
                                    nc.vector.tensor_add(h_new[:, co, sl], gt[:],
                                                         pred[:, co, sl])
                            if i + 1 == n_heads:
                                # fc_out + pose for this s-half right away
                                if half == 0:
                                    _mark(nc, "fc_out")
                                    if with_bias:
                                        fob_sb = small.tile([1, C], F8, tag="fob")
                                        nc.sync.dma_start(fob_sb[:], fob[:])
                                for ss in range(half * 4, half * 4 + 4):
                                    ps = psA.tile([128, C], F32, tag="mmA")
                                    _mm_full(
                                        nc, ps,
                                        lambda kk, ss=ss: h_new[:, kk:kk + 2, ss * 128:(ss + 1) * 128],
                                        lambda kk, sl2: fow_sb[:, kk:kk + 2, sl2], KT,
                                        extra=(lambda sl2: nc.tensor.matmul(
                                            ps[:, sl2], ones_row8[:], fob_sb[0:1, sl2],
                                            start=False, stop=True)) if with_bias else None)
                                    o_sb = fo2.tile([128, C], F32, tag="osb")
                                    nc.vector.scalar_tensor_tensor(
                                        o_sb[:], ps[:], 1.0 / WS, pe_sb[:, ss, :],
                                        op0=MULT, op1=mybir.AluOpType.add)
                                    nc.sync.dma_start(out[ss * 128:(ss + 1) * 128, :], o_sb[:])
                        h_own = h_new
                        if i + 1 < n_heads:
                            wq_sb, wk_sb, wv_sb, wp_sb = wq_n, wk_n, wv_n, wp_n


    nc.compile()
    return nc


def build_null() -> bacc.Bacc:
    """Same I/O signature, ~no compute: measures the dispatch floor."""
    nc = bacc.Bacc(num_devices=N_CORES, name="attn_null")
    nc.dram_tensor("x_t", [128, KT, S_OWN], F8, kind="ExternalInput")
    nc.dram_tensor("fc_in_wT", [128, KT, C], F8, kind="ExternalInput")
    nc.dram_tensor("fc_in_b_row", [1, C], F8, kind="ExternalInput")
    nc.dram_tensor("ln_g_row", [1, C], F32, kind="ExternalInput")
    nc.dram_tensor("ln_b_row", [1, C], F32, kind="ExternalInput")
    nc.dram_tensor("wq_t", [H, 128, KT, C], F8, kind="ExternalInput")
    nc.dram_tensor("wk_t", [H, 128, KT, C], F8, kind="ExternalInput")
    nc.dram_tensor("wv_t", [H, 128, KT, C], F8, kind="ExternalInput")
    nc.dram_tensor("wp_t", [H, 128, KT, C], F8, kind="ExternalInput")
    nc.dram_tensor("q_b_col", [H, 128, KT], F32, kind="ExternalInput")
    nc.dram_tensor("k_b_row", [H, 1, C], F8, kind="ExternalInput")
    nc.dram_tensor("v_b_row", [H, 1, C], F8, kind="ExternalInput")
    nc.dram_tensor("proj_b_col", [H, 128, KT], F32, kind="ExternalInput")
    nc.dram_tensor("fc_out_wT", [128, KT, C], F8, kind="ExternalInput")
    nc.dram_tensor("fc_out_b_row", [1, C], F8, kind="ExternalInput")
    nc.dram_tensor("offs", [1, 2], I32, kind="ExternalInput")
    nc.dram_tensor("can_in", [H, 64], F8, kind="ExternalInput")
    pe = nc.dram_tensor("pe", [S_OWN, C], BF16, kind="ExternalInput")
    out = nc.dram_tensor("out", [S_OWN, C], F32, kind="ExternalOutput")
    nc.dram_tensor("canary_out", [H, 64], F8, kind="ExternalOutput")
    with tile.TileContext(nc) as tc:
        with tc.tile_pool(name="p", bufs=2) as p:
            for ss in range(NT_OWN):
                t = p.tile([128, C], BF16, tag="t")
                nc.sync.dma_start(t[:], pe[ss * 128:(ss + 1) * 128, :])
                t2 = p.tile([128, C], F32, tag="t2")
                nc.vector.tensor_copy(t2[:], t[:])
                nc.sync.dma_start(out[ss * 128:(ss + 1) * 128, :], t2[:])
    nc.compile()
    return nc


def _pose_enc_np(s, f):
    pos = np.arange(s, dtype=np.float32)[:, None]
    div = (1.0 / (1000.0 ** (2.0 * np.arange(f, dtype=np.float32) / np.float32(f))))[None, :]
    p = np.zeros((s, f), np.float32)
    p[0::2, :] = np.sin(pos[0::2] * div)
    p[1::2, :] = np.cos(pos[1::2] * div)
    return p


def _f8(a, scale=1.0):
    return np.ascontiguousarray((np.asarray(a, np.float32) * scale).astype(NP8))


def _tile_kt(mat):
    """[C_in, N] -> [128, KT, N] (c_in = kt*128 + partition)."""
    cin, n = mat.shape
    return np.ascontiguousarray(mat.reshape(KT, 128, n).transpose(1, 0, 2))


def prepare_in_maps(x, fc_in_w, fc_in_b, ln_g, ln_b, qkv_w, qkv_b, proj_w, proj_b,
                    fc_out_w, fc_out_b):
    x = np.asarray(x, np.float32)
    qkv_w = np.asarray(qkv_w, np.float32)
    qkv_b = np.asarray(qkv_b, np.float32)
    proj_w = np.asarray(proj_w, np.float32)
    proj_b = np.asarray(proj_b, np.float32)

    shared = {
        "fc_in_wT": _tile_kt(_f8(np.asarray(fc_in_w, np.float32).T, WS)),
        "fc_in_b_row": _f8(np.asarray(fc_in_b)[None, :], WS),
        "ln_g_row": np.ascontiguousarray(np.asarray(ln_g, np.float32)[None, :]),
        "ln_b_row": np.ascontiguousarray(np.asarray(ln_b, np.float32)[None, :]),
        "wq_t": np.stack([_tile_kt(_f8(qkv_w[i, 0:C, :].T, WS)) for i in range(H)]),
        "wk_t": np.stack([_tile_kt(_f8(qkv_w[i, C:2 * C, :].T, WS)) for i in range(H)]),
        "wv_t": np.stack([_tile_kt(_f8(qkv_w[i, 2 * C:, :].T, WS)) for i in range(H)]),
        "wp_t": np.stack([_tile_kt(_f8(proj_w[i].T, WS)) for i in range(H)]),
        "q_b_col": np.ascontiguousarray(
            qkv_b[:, 0:C].reshape(H, KT, 128).transpose(0, 2, 1)),
        "k_b_row": _f8(qkv_b[:, C:2 * C][:, None, :], WS),
        "v_b_row": _f8(qkv_b[:, 2 * C:][:, None, :], WS),
        "proj_b_col": np.ascontiguousarray(
            proj_b.reshape(H, KT, 128).transpose(0, 2, 1)),
        "fc_out_wT": _tile_kt(_f8(np.asarray(fc_out_w, np.float32).T, WS)),
        "fc_out_b_row": _f8(np.asarray(fc_out_b)[None, :], WS),
    }
    pe_full = _pose_enc_np(S, C)
    in_maps = []
    for core in range(N_CORES):
        b, half = divmod(core, 2)
        own = x[b, half * S_OWN:(half + 1) * S_OWN, :].T  # [C, S_OWN]
        m = dict(shared)
        m["x_t"] = _tile_kt(_f8(own))
        m["pe"] = np.ascontiguousarray(
            pe_full[half * S_OWN:(half + 1) * S_OWN, :].astype(NPBF))
        slot_elems = 128 * (KT * C + 64)
        m["offs"] = np.array([[half * slot_elems, (1 - half) * slot_elems]], np.int32)
        m["can_in"] = np.stack(
            [np.full((64,), float((core + 1) * (2 ** i)), NP8) for i in range(H)])
        in_maps.append(m)
    return in_maps


_NC_CACHE = {}


def get_nc(n_heads=H, with_bias=True, with_ln_affine=True):
    key = (n_heads, with_bias, with_ln_affine)
    if key not in _NC_CACHE:
        _NC_CACHE[key] = build(n_heads, with_bias, with_ln_affine)
    return _NC_CACHE[key]


_EXEC_CACHE = {}


def _get_executable(nc):
    """One jitted collectives executable per process (loading a second one
    hangs the axon worker); reused across kernel() calls."""
    key = id(nc)
    if key in _EXEC_CACHE:
        return _EXEC_CACHE[key]
    import jax
    from jax.sharding import Mesh, PartitionSpec, NamedSharding
    from jax.experimental.shard_map import shard_map
    from concourse import bass2jax
    import concourse.mybir as mybir_

    bass2jax.install_neuronx_cc_hook()
    partition_name = nc.partition_id_tensor.name if nc.partition_id_tensor else None
    in_names, out_names, out_avals, zero_outs = [], [], [], []
    for alloc in nc.m.functions[0].allocations:
        if not isinstance(alloc, mybir_.MemoryLocationSet):
            continue
        name = alloc.memorylocations[0].name
        if alloc.kind == "ExternalInput":
            if name != partition_name:
                in_names.append(name)
        elif alloc.kind == "ExternalOutput":
            out_names.append(name)
            shape = tuple(alloc.tensor_shape)
            dtype = mybir_.dt.np(alloc.dtype)
            out_avals.append(jax.core.ShapedArray(shape, dtype))
            zero_outs.append(np.zeros(shape, dtype))
    n_params = len(in_names)
    n_outs = len(out_avals)
    all_in = in_names + out_names + ([partition_name] if partition_name else [])
    donate = tuple(range(n_params, n_params + n_outs))

    def _body(*args):
        operands = list(args)
        if partition_name is not None:
            operands.append(bass2jax.partition_id_tensor())
        return tuple(bass2jax._bass_exec_p.bind(
            *operands, out_avals=tuple(out_avals), in_names=tuple(all_in),
            out_names=tuple(out_names), lowering_input_output_aliases=(),
            sim_require_finite=True, sim_require_nnan=True, nc=nc))

    devices = jax.devices()[:N_CORES]
    mesh = Mesh(np.asarray(devices), ("core",))
    sharded = jax.jit(
        shard_map(_body, mesh=mesh,
                  in_specs=(PartitionSpec("core"),) * (n_params + n_outs),
                  out_specs=(PartitionSpec("core"),) * len(out_names),
                  check_rep=False),
        donate_argnums=donate, keep_unused=True)
    sh = NamedSharding(mesh, PartitionSpec("core"))
    entry = (sharded, sh, in_names[:n_params], out_names, out_avals, zero_outs)
    _EXEC_CACHE[key] = entry
    return entry


def flags_for(inputs):
    with_bias = not (np.all(np.asarray(inputs["fc_in_b"]) == 0)
                     and np.all(np.asarray(inputs["qkv_b"]) == 0)
                     and np.all(np.asarray(inputs["proj_b"]) == 0)
                     and np.all(np.asarray(inputs["fc_out_b"]) == 0))
    with_ln = not (np.all(np.asarray(inputs["ln_g"]) == 1)
                   and np.all(np.asarray(inputs["ln_b"]) == 0))
    return with_bias, with_ln


_WARMED = set()


def canaries_ok(out_arrs, out_names, out_avals):
    """True iff every core read its partner's per-head canary in every head:
    proves each head's exchange (including the early-barrier race) was clean
    for that execution."""
    ci = out_names.index("canary_out")
    pc = np.asarray(out_arrs[ci]).reshape(N_CORES, *out_avals[ci].shape)
    vals = pc.astype(np.float32)
    for core in range(N_CORES):
        for i in range(vals.shape[1]):
            if not np.all(vals[core, i] == float(((core ^ 1) + 1) * (2 ** i))):
                return False
    return True


def kernel(**inputs) -> np.ndarray:
    with_bias, with_ln = flags_for(inputs)
    nc = get_nc(H, with_bias, with_ln)
    in_maps = prepare_in_maps(**inputs)
    import jax
    sharded, sh, in_names, out_names, out_avals, zero_outs = _get_executable(nc)
    concat_in = [jax.device_put(
        np.concatenate([np.asarray(in_maps[c][nm]) for c in range(N_CORES)], axis=0), sh)
        for nm in in_names]

    def one_call():
        concat_zeros = [jax.device_put(
            np.zeros((N_CORES * z.shape[0], *z.shape[1:]), z.dtype), sh)
            for z in zero_outs]
        out_arrs = sharded(*concat_in, *concat_zeros)
        jax.block_until_ready(out_arrs)
        return out_arrs

    # The first executions after NEFF load race DGE descriptor generation
    # against the dynamic-offset register loads; registers persist across
    # executions, so retry until the canaries prove the exchange addressed
    # the right slots (typically clean from the 2nd execution).
    if id(nc) not in _WARMED:
        one_call()
        _WARMED.add(id(nc))
    for _attempt in range(8):
        out_arrs = one_call()
        oi_ = out_names.index("out")
        pc_ = np.asarray(out_arrs[oi_])
        if canaries_ok(out_arrs, out_names, out_avals) and not np.isnan(pc_).any():
            break
    oi = out_names.index("out")
    per_core = np.asarray(out_arrs[oi]).reshape(N_CORES, *out_avals[oi].shape)
    out_full = np.empty((B, S, C), np.float32)
    for core in range(N_CORES):
        b, half = divmod(core, 2)
        out_full[b, half * S_OWN:(half + 1) * S_OWN, :] = per_core[core]
    return out_full


# revision 7
# speedup vs baseline: 1.3627x; 1.0734x over previous
"""Distributed Trainium2 kernel for nn_Attention_18562848653411 (v2).

Reference model: fc_in -> LayerNorm -> 4 sequential "refinement heads"
(qkv matmul + gelu, scores=q@k^T/C, att=scores@v, softmax over channels,
proj + gelu, residual with head-0 output) -> fc_out + PoseEncoding.

Key algebra: softmax comes AFTER att = scores@v, so per head
att^T = (v^T k) q^T / C = M q^T / C with M = v^T k a [C,C] matrix that
is a sum over sequence positions. No S x S scores are ever formed.

Sharding (8 NeuronCores): core c handles batch b=c//2, sequence half
c%2. All weights replicated, fp8e4 (x16 host scale); every big matmul is
a DoubleRow fp8 instruction. Per head each core computes k/v and
M_own = v_own^T k_own over its OWN 1024 rows and exchanges M_own (1 MB
fp8) with its pair partner through pair-shared HBM. The pair barrier
(tiny AllGather) is issued EARLY (right after k/v) so its ~15us fixed
latency overlaps the M matmuls and M writes; per-head canary values
(core+1)*2^head written after the M payload on the same queue witness
that the partner's writes landed before our read -- kernel() retries
until canaries prove a clean exchange.

Softmax over channels runs on transposed tiles att^T[c, s]: exp via
activation (scale=1/16, bias=-8ln2), DoubleRow ones-matmul denominator,
gpsimd partition-broadcast of 256/denom, DVE normalize; proj descales by
1/(16*256) inside its gelu. The apply step accumulates m_own q and
m_partner q directly in PSUM (no pre-add of the M halves). LayerNorm
runs in row space off the fc_in PSUM (bn_stats, Act does only Sqrt so a
single act table serves all of fc_in); h^T comes from PE transposes +
Act Copy (no DMA transposes).
"""

import numpy as np
import ml_dtypes

import concourse.bass as bass
import concourse.mybir as mybir
import concourse.tile as tile
from concourse import bacc
from concourse.bass_utils import run_bass_kernel_spmd  # noqa: F401
from concourse.masks import make_identity

N_CORES = 8
PAIRS = [[0, 1], [2, 3], [4, 5], [6, 7]]
B, S, C = 4, 2048, 1024
H = 4
S_OWN = S // 2
KT = C // 128          # 8 contraction tiles of 128
NT_OWN = S_OWN // 128  # 8 own t tiles
HALF = 512

F32 = mybir.dt.float32
BF16 = mybir.dt.bfloat16
F8 = mybir.dt.float8e4
I32 = mybir.dt.int32
GELU = mybir.ActivationFunctionType.Gelu
EXP = mybir.ActivationFunctionType.Exp
SQRT = mybir.ActivationFunctionType.Sqrt
COPY = mybir.ActivationFunctionType.Copy
SUB = mybir.AluOpType.subtract
MULT = mybir.AluOpType.mult
BYPASS = mybir.AluOpType.bypass
DR = mybir.MatmulPerfMode.DoubleRow

NP8 = ml_dtypes.float8_e4m3fn
NPBF = ml_dtypes.bfloat16

WS = 16.0             # host weight scale
MSC = 64.0            # M stored as M/MSC
APS = C / MSC         # apply psum = APS * att_raw = 16*att
EXP_SHIFT = 8.0       # exp output scaled 2^-8
NORM_SCALE = 256.0    # normalized att stored x256
M_ELEMS = 128 * KT * C  # one M half (1 MB fp8)
GATE_TT = 1  # v tile whose completion launches the pair barrier

PHASE_MARKS = []


def _mark(nc, name):
    PHASE_MARKS.append((name, int(nc.get_next_instruction_name().split("-")[1])))


def _mm_full(nc, ps, lhsT_of, rhs_of, n_k, extra=None):
    """Accumulate a [128, 1024] psum tile in two 512-col bank halves with
    DoubleRow fp8 matmuls. lhsT_of(kk) -> [128,2,128]; rhs_of(kk, sl) ->
    [128,2,512]. extra(sl) appends a bias matmul closing the group."""
    for half in range(2):
        sl = slice(half * HALF, (half + 1) * HALF)
        for kk in range(0, n_k, 2):
            nc.tensor.matmul(ps[:, sl], lhsT_of(kk), rhs_of(kk, sl),
                             start=(kk == 0),
                             stop=(extra is None and kk == n_k - 2),
                             perf_mode=DR)
        if extra is not None:
            extra(sl)


def build(n_heads: int = H, with_bias: bool = True, with_ln_affine: bool = True) -> bacc.Bacc:
    PHASE_MARKS.clear()
    nc = bacc.Bacc(num_devices=N_CORES, name="attn")

    x_t = nc.dram_tensor("x_t", [128, KT, S_OWN], F8, kind="ExternalInput")
    fcw = nc.dram_tensor("fc_in_wT", [128, KT, C], F8, kind="ExternalInput")
    fcb = nc.dram_tensor("fc_in_b_row", [1, C], F8, kind="ExternalInput")
    lng = nc.dram_tensor("ln_g_row", [1, C], F32, kind="ExternalInput")
    lnb = nc.dram_tensor("ln_b_row", [1, C], F32, kind="ExternalInput")
    wq = nc.dram_tensor("wq_t", [H, 128, KT, C], F8, kind="ExternalInput")
    wk = nc.dram_tensor("wk_t", [H, 128, KT, C], F8, kind="ExternalInput")
    wv = nc.dram_tensor("wv_t", [H, 128, KT, C], F8, kind="ExternalInput")
    wp = nc.dram_tensor("wp_t", [H, 128, KT, C], F8, kind="ExternalInput")
    qb = nc.dram_tensor("q_b_col", [H, 128, KT], F32, kind="ExternalInput")
    kb = nc.dram_tensor("k_b_row", [H, 1, C], F8, kind="ExternalInput")
    vb = nc.dram_tensor("v_b_row", [H, 1, C], F8, kind="ExternalInput")
    pb = nc.dram_tensor("proj_b_col", [H, 128, KT], F32, kind="ExternalInput")
    fow = nc.dram_tensor("fc_out_wT", [128, KT, C], F8, kind="ExternalInput")
    fob = nc.dram_tensor("fc_out_b_row", [1, C], F8, kind="ExternalInput")
    offs = nc.dram_tensor("offs", [1, 2], I32, kind="ExternalInput")
    can_in = nc.dram_tensor("can_in", [H, 64], F8, kind="ExternalInput")
    pe = nc.dram_tensor("pe", [S_OWN, C], BF16, kind="ExternalInput")
    out = nc.dram_tensor("out", [S_OWN, C], F32, kind="ExternalOutput")
    canary_out = nc.dram_tensor("canary_out", [H, 64], F8, kind="ExternalOutput")

    ROWE = KT * C + 64  # row stride: M payload + canary pad
    hsh = [nc.dram_tensor(f"hsh{i}", [2, 128, ROWE], F8,
                          kind="Internal", addr_space="Shared")
           for i in range(n_heads)]
    bar_in = nc.dram_tensor("bar_in", [1, 3], F8, kind="Internal")
    bar_out = [nc.dram_tensor(f"bar_out{i}", [2, 3], F8, kind="Internal")
               for i in range(n_heads)]

    with tile.TileContext(nc) as tc:
        with (
            tc.tile_pool(name="pers", bufs=1) as pers,
            tc.tile_pool(name="hpool", bufs=2) as hpool,
            tc.tile_pool(name="wpool", bufs=2) as wpool,
            tc.tile_pool(name="small", bufs=2) as small,
            tc.tile_pool(name="act", bufs=2) as actp,
        ):
            ones16 = pers.tile([128, 2, 16], F8)
            nc.vector.memset(ones16[:], 1.0)
            if with_bias:
                ones_row8 = pers.tile([1, 128], F8)
                nc.vector.memset(ones_row8[:], 1.0)
            eps_t = pers.tile([128, 1], F32)
            nc.vector.memset(eps_t[:], 1e-5)
            expb = pers.tile([128, 1], F32)
            nc.vector.memset(expb[:], -float(EXP_SHIFT) * float(np.log(2.0)))
            ident = pers.tile([128, 128], BF16)
            make_identity(nc, ident[:])
            pred = pers.tile([128, KT, S_OWN], F8, name="pred")
            pe_sb = pers.tile([128, NT_OWN, C], BF16, name="pe_sb")

            off_sb = pers.tile([1, 2], I32)
            nc.sync.dma_start(off_sb[:], offs[:])
            r_w = nc.sync.alloc_register("r_w")
            r_r = nc.sync.alloc_register("r_r")
            nc.sync.reg_load(r_w, off_sb[0:1, 0:1])
            nc.sync.reg_load(r_r, off_sb[0:1, 1:2])
            r_wc = nc.sync.alloc_register("r_wc")
            r_rc = nc.sync.alloc_register("r_rc")
            r_w2 = nc.sync.alloc_register("r_w2")
            r_r2 = nc.sync.alloc_register("r_r2")
            nc.sync.reg_add(r_wc, r_w, KT * C)
            nc.sync.reg_add(r_rc, r_r, KT * C)
            nc.sync.reg_add(r_w2, r_w, KT * C // 2)
            nc.sync.reg_add(r_r2, r_r, KT * C // 2)
            can_sb = pers.tile([H, 64], F8)
            nc.sync.dma_start(can_sb[:], can_in[:])

            # head-0 weights prefetch on gpsimd (SWDGE; no HWDGE contention);
            # wk/wv first -- the fc_in loop interleaves head-0 k/v matmuls
            wk_sb = wpool.tile([128, KT, C], F8, tag="wk")
            nc.gpsimd.dma_start(wk_sb[:], wk[0])
            wv_sb = wpool.tile([128, KT, C], F8, tag="wv")
            nc.gpsimd.dma_start(wv_sb[:], wv[0])
            wq_sb = wpool.tile([128, KT, C], F8, tag="wq")
            nc.gpsimd.dma_start(wq_sb[:], wq[0])
            wp_sb = wpool.tile([128, KT, C], F8, tag="wp")
            nc.gpsimd.dma_start(wp_sb[:], wp[0])
            if with_bias:
                kb_sb = small.tile([1, C], F8, tag="kb")
                nc.sync.dma_start(kb_sb[:], kb[0])
                vb_sb = small.tile([1, C], F8, tag="vb")
                nc.sync.dma_start(vb_sb[:], vb[0])
            k0_sb = None  # head-0 k/v tiles, filled by the fc_in loop

            # ================= fc_in + LayerNorm (own rows only) ============
            _mark(nc, "fc_in")
            h_own = hpool.tile([128, KT, S_OWN], F8, tag="hT", name="hT0")
            k0_sb = pers.tile([128, NT_OWN, C], F8, name="k0_sb")
            v0_sb = pers.tile([128, NT_OWN, C], F8, name="v0_sb")
            with (tc.tile_pool(name="s0", bufs=1) as s0,
                  tc.tile_pool(name="s0ps", bufs=3, space="PSUM") as s0ps,
                  tc.tile_pool(name="s0tp", bufs=2, space="PSUM") as s0tp):
                fcw_sb = s0.tile([128, KT, C], F8)
                nc.sync.dma_start(fcw_sb[:], fcw[:])
                x_sb = s0.tile([128, KT, S_OWN], F8)
                nc.sync.dma_start(x_sb[:, :, 0:HALF], x_t[:, :, 0:HALF])
                nc.sync.dma_start(x_sb[:, :, HALF:S_OWN], x_t[:, :, HALF:S_OWN])
                if with_bias:
                    fcb_sb = s0.tile([1, C], F8)
                    nc.sync.dma_start(fcb_sb[:], fcb[:])
                if with_ln_affine:
                    g_bc = s0.tile([128, C], F32)
                    nc.sync.dma_start(g_bc[:], bass.AP(tensor=lng, offset=0,
                                                       ap=[[0, 128], [1, C]]))
                    b_bc = s0.tile([128, C], F32)
                    nc.sync.dma_start(b_bc[:], bass.AP(tensor=lnb, offset=0,
                                                       ap=[[0, 128], [1, C]]))
                for blk in range(1):
                    # 4 fc_in tiles (Act: sqrt+copy, one table); the PE
                    # transposes trail the mm/LN chain by one tile so the next
                    # matmul never waits on the DVE normalize
                    hnb_q = []

                    def _transp(ss, hnb):
                        for grp in range(2):
                            tp = s0tp.tile([128, 4, 128], BF16, tag="tp")
                            for j in range(4):
                                cc = grp * 4 + j
                                nc.tensor.transpose(
                                    tp[:, j, :], hnb[:, cc * 128:(cc + 1) * 128], ident[:])
                            nc.scalar.activation(
                                h_own[:, grp * 4:(grp + 1) * 4, ss * 128:(ss + 1) * 128],
                                tp[:], COPY)

                    for ss in range(8):
                        ps = s0ps.tile([128, C], F32, tag="mmA")
                        _mm_full(
                            nc, ps,
                            lambda kk, ss=ss: x_sb[:, kk:kk + 2, ss * 128:(ss + 1) * 128],
                            lambda kk, sl: fcw_sb[:, kk:kk + 2, sl], KT,
                            extra=(lambda sl: nc.tensor.matmul(
                                ps[:, sl], ones_row8[:], fcb_sb[0:1, sl],
                                start=False, stop=True)) if with_bias else None)
                        stats = small.tile([128, 2, 6], F32, tag="bnst")
                        nc.vector.bn_stats(stats[:, 0, :], ps[:, 0:HALF])
                        nc.vector.bn_stats(stats[:, 1, :], ps[:, HALF:C])
                        mv = small.tile([128, 2], F32, tag="mv")
                        nc.vector.bn_aggr(mv[:], stats[:])
                        rstd = small.tile([128, 1], F32, tag="rstd")
                        nc.scalar.activation(rstd[:], mv[:, 1:2], SQRT, bias=eps_t[:], scale=1.0)
                        nc.vector.reciprocal(rstd[:], rstd[:])
                        hnb = s0.tile([128, C], BF16, tag="hnb", bufs=3)
                        nc.vector.tensor_scalar(hnb[:], ps[:], mv[:, 0:1], rstd[:],
                                                op0=SUB, op1=MULT)
                        if with_ln_affine:
                            hnf = s0.tile([128, C], F32, tag="hnf", bufs=2)
                            nc.vector.tensor_mul(hnf[:], hnb[:], g_bc[:])
                            nc.vector.tensor_add(hnb[:], hnf[:], b_bc[:])
                        hnb_q.append((ss, hnb))
                        if len(hnb_q) > 1:
                            _transp(*hnb_q.pop(0))
                    while hnb_q:
                        _transp(*hnb_q.pop(0))
                    # ... then head-0 k/v for those tiles (Act: gelu) -- block
                    # granularity keeps act-table transitions to one per block
                    for tt in range(8):
                        for dst, wmat, bias_sb in ((k0_sb, wk_sb, kb_sb if with_bias else None),
                                                   (v0_sb, wv_sb, vb_sb if with_bias else None)):
                            ps = s0ps.tile([128, C], F32, tag="mmA")
                            _mm_full(
                                nc, ps,
                                lambda kk, tt=tt: h_own[:, kk:kk + 2, tt * 128:(tt + 1) * 128],
                                lambda kk, sl, wmat=wmat: wmat[:, kk:kk + 2, sl], KT,
                                extra=(lambda sl, b=bias_sb: nc.tensor.matmul(
                                    ps[:, sl], ones_row8[:], b[0:1, sl],
                                    start=False, stop=True)) if with_bias else None)
                            nc.scalar.activation(dst[:, tt, :], ps[:], GELU, scale=1.0 / WS)

            # (gelu table load happens at head 0's first kv gelu)

            # ================= heads =================
            with (tc.tile_pool(name="psA", bufs=2, space="PSUM") as psA,
                  tc.tile_pool(name="psH", bufs=3, space="PSUM") as psH,
                  tc.tile_pool(name="psD", bufs=1, space="PSUM") as psD,
                  tc.tile_pool(name="fo2", bufs=2) as fo2):
                for i in range(n_heads):
                    wi = i % H
                    _mark(nc, f"head{i}")
                    with tc.tile_pool(name=f"hd{i}", bufs=1) as hp:
                        if with_bias:
                            if i > 0:
                                kb_sb = small.tile([1, C], F8, tag="kb")
                                nc.sync.dma_start(kb_sb[:], kb[wi])
                                vb_sb = small.tile([1, C], F8, tag="vb")
                                nc.sync.dma_start(vb_sb[:], vb[wi])
                            qb_sb = small.tile([128, KT], F32, tag="qb")
                            nc.sync.dma_start(qb_sb[:], qb[wi])
                            pb_sb = small.tile([128, KT], F32, tag="pb")
                            nc.sync.dma_start(pb_sb[:], pb[wi])

                        q_sb = hp.tile([128, KT, S_OWN], F8, name="q_sb")
                        if i == 0:
                            k_sb, v_sb = k0_sb, v0_sb
                        else:
                            k_sb = hp.tile([128, NT_OWN, C], F8, name="k_sb")
                            v_sb = hp.tile([128, NT_OWN, C], F8, name="v_sb")
                        m_sb = hp.tile([128, KT, C], F8, name="m_sb")
                        mp_sb = hp.tile([128, KT, C], F8, name="mp_sb")
                        attsm = hp.tile([128, KT, S_OWN], F8, name="attsm")

                        # ---- k, v [t, c] (h-stationary) over own rows
                        # (head 0's k/v were interleaved into the fc_in loop)
                        if i > 0:
                            kv_list = ((k_sb, wk_sb, kb_sb if with_bias else None),
                                       (v_sb, wv_sb, vb_sb if with_bias else None))
                            for dst, wmat, bias_sb in kv_list:
                                for tt in range(NT_OWN):
                                    ps = psA.tile([128, C], F32, tag="mmA")
                                    _mm_full(
                                        nc, ps,
                                        lambda kk, tt=tt: h_own[:, kk:kk + 2, tt * 128:(tt + 1) * 128],
                                        lambda kk, sl, wmat=wmat: wmat[:, kk:kk + 2, sl], KT,
                                        extra=(lambda sl, b=bias_sb: nc.tensor.matmul(
                                            ps[:, sl], ones_row8[:], b[0:1, sl],
                                            start=False, stop=True)) if with_bias else None)
                                    nc.scalar.activation(dst[:, tt, :], ps[:], GELU, scale=1.0 / WS)

                        # ---- early pair barrier: gate bar_in on v's last tile
                        # so the ~15us collective overlaps the M matmuls and
                        # M writes (canaries verify the race was won).
                        _mark(nc, f"h{i}_bar")
                        # barrier payload is irrelevant; source it from v tile 3
                        # so the collective launches once v is half done (the
                        # remaining ~15us of barrier covers M compute + writes
                        # on both cores; canaries verify the race was won)
                        nc.sync.dma_start(bar_in[:], v_sb[0:1, GATE_TT, 0:3])
                        nc.gpsimd.collective_compute(
                            "AllGather", BYPASS, replica_groups=PAIRS,
                            ins=[bar_in[:].opt()], outs=[bar_out[i][:].opt()])

                        # next-head weight prefetch (gpsimd queue, after the
                        # collective so transfers run under the barrier)
                        if i + 1 < n_heads:
                            nwi = (i + 1) % H
                            wq_n = wpool.tile([128, KT, C], F8, tag="wq")
                            nc.gpsimd.dma_start(wq_n[:], wq[nwi])
                            wk_n = wpool.tile([128, KT, C], F8, tag="wk")
                            nc.gpsimd.dma_start(wk_n[:], wk[nwi])
                            wv_n = wpool.tile([128, KT, C], F8, tag="wv")
                            nc.gpsimd.dma_start(wv_n[:], wv[nwi])
                            wp_n = wpool.tile([128, KT, C], F8, tag="wp")
                            nc.gpsimd.dma_start(wp_n[:], wp[nwi])
                        if i == 1:
                            nc.gpsimd.dma_start(
                                pe_sb[:], bass.AP(tensor=pe, offset=0,
                                                  ap=[[C, 128], [128 * C, NT_OWN], [1, C]]))
                        if i == 2:
                            fow_sb = pers.tile([128, KT, C], F8, name="fow_sb")
                            nc.gpsimd.dma_start(fow_sb[:], fow[:])

                        # ---- M_own = v_own^T k_own (x 1/MSC), [c, cq]
                        _mark(nc, f"h{i}_M")
                        for co in range(KT):
                            ps = psA.tile([128, C], F32, tag="mmA")
                            _mm_full(
                                nc, ps,
                                lambda tt, co=co: v_sb[:, tt:tt + 2, co * 128:(co + 1) * 128],
                                lambda tt, sl: k_sb[:, tt:tt + 2, sl], NT_OWN)
                            nc.vector.tensor_scalar_mul(m_sb[:, co, :], ps[:], 1.0 / MSC)
                            if co == KT // 2 - 1:
                                wap = bass.AP(tensor=hsh[i], offset=r_w,
                                              ap=[[ROWE, 128], [1, KT * C // 2]],
                                              dep_tracking_offset=0)
                                nc.sync.dma_start(wap, m_sb[:, 0:KT // 2, :])
                        wap2 = bass.AP(tensor=hsh[i], offset=r_w2,
                                       ap=[[ROWE, 128], [1, KT * C // 2]],
                                       dep_tracking_offset=KT * C // 2)
                        nc.sync.dma_start(wap2, m_sb[:, KT // 2:KT, :])
                        wcap = bass.AP(tensor=hsh[i], offset=r_wc, ap=[[64, 1], [1, 64]],
                                       dep_tracking_offset=KT * C)
                        nc.sync.dma_start(wcap, can_sb[wi:wi + 1, :])

                        # ---- q [co, s] (w-stationary) -- fills barrier window
                        _mark(nc, f"h{i}_q")
                        for co in range(KT):
                            ps = psA.tile([128, S_OWN], F32, tag="mmA")
                            _mm_full(
                                nc, ps,
                                lambda kk, co=co: wq_sb[:, kk:kk + 2, co * 128:(co + 1) * 128],
                                lambda kk, sl: h_own[:, kk:kk + 2, sl], KT)
                            if with_bias:
                                nc.scalar.activation(q_sb[:, co, :], ps[:], GELU,
                                                     bias=qb_sb[:, co:co + 1], scale=1.0 / WS)
                            else:
                                nc.scalar.activation(q_sb[:, co, :], ps[:], GELU,
                                                     scale=1.0 / WS)

                        # (exp table load happens at the first apply exp)

                        # ---- barrier done: canary first, then partner M
                        bar_sb = hp.tile([2, 3], F8, name="bar_sb")
                        nc.sync.dma_start(bar_sb[:], bar_out[i][:])
                        rcap = bass.AP(tensor=hsh[i], offset=r_rc, ap=[[64, 1], [1, 64]],
                                       dep_tracking_offset=M_ELEMS + KT * C)
                        can_rd = hp.tile([1, 64], F8, name="can_rd")
                        nc.sync.dma_start(can_rd[:], rcap)
                        nc.sync.dma_start(canary_out[wi:wi + 1, :], can_rd[:])
                        rap0 = bass.AP(tensor=hsh[i], offset=r_r,
                                       ap=[[ROWE, 128], [1, KT * C // 2]],
                                       dep_tracking_offset=M_ELEMS)
                        nc.sync.dma_start(mp_sb[:, 0:KT // 2, :], rap0)
                        rap1 = bass.AP(tensor=hsh[i], offset=r_r2,
                                       ap=[[ROWE, 128], [1, KT * C // 2]],
                                       dep_tracking_offset=M_ELEMS + KT * C // 2)
                        nc.sync.dma_start(mp_sb[:, KT // 2:KT, :], rap1)

                        # ---- apply + exp, per s-half. Half 0 accumulates m
                        # and mp straight in PSUM (starts when the partner
                        # chunk lands); half 1 uses the DVE pre-added msum
                        # (computed in half 0's shadow), halving its matmuls.
                        _mark(nc, f"h{i}_apply")
                        msum = hp.tile([128, KT, C], F8, name="msum")
                        for half in range(2):
                            sl = slice(half * HALF, (half + 1) * HALF)
                            for co in range(KT):
                                ps = psH.tile([128, HALF], F32, tag="mm5")
                                if half == 0:
                                    for src_i, msrc in enumerate((m_sb, mp_sb)):
                                        for cc in range(0, KT, 2):
                                            nc.tensor.matmul(
                                                ps[:],
                                                msrc[:, cc:cc + 2, co * 128:(co + 1) * 128],
                                                q_sb[:, cc:cc + 2, sl],
                                                start=(src_i == 0 and cc == 0),
                                                stop=(src_i == 1 and cc == KT - 2),
                                                perf_mode=DR)
                                else:
                                    for cc in range(0, KT, 2):
                                        nc.tensor.matmul(
                                            ps[:],
                                            msum[:, cc:cc + 2, co * 128:(co + 1) * 128],
                                            q_sb[:, cc:cc + 2, sl],
                                            start=(cc == 0), stop=(cc == KT - 2),
                                            perf_mode=DR)
                                nc.scalar.activation(attsm[:, co, sl], ps[:], EXP,
                                                     bias=expb[:], scale=1.0 / APS)
                            if half == 0:
                                # msum for half 1, under half 0's matmuls
                                for cc in range(KT):
                                    nc.vector.tensor_add(msum[:, cc, :], m_sb[:, cc, :],
                                                         mp_sb[:, cc, :])
                            # denominator (DoubleRow ones) + 256/denom
                            # broadcast, pipelined per s-quarter so proj can
                            # chase the normalize front
                            for qq in range(2):
                                qsl = slice(half * HALF + qq * 256,
                                            half * HALF + (qq + 1) * 256)
                                dn = psD.tile([16, 256], F32, tag="dn")
                                for cc in range(0, KT, 2):
                                    nc.tensor.matmul(dn[:], ones16[:],
                                                     attsm[:, cc:cc + 2, qsl],
                                                     start=(cc == 0), stop=(cc == KT - 2),
                                                     perf_mode=DR)
                                rr = small.tile([1, 256], F32, tag="rr")
                                nc.vector.reciprocal(rr[:], dn[0:1, :])
                                rr2 = small.tile([1, 256], F32, tag="rr2")
                                nc.vector.tensor_scalar_mul(rr2[:], rr[:], NORM_SCALE)
                                bc = small.tile([128, 256], F32, tag="bc")
                                nc.gpsimd.partition_broadcast(bc[:], rr2[0:1, :], channels=128)
                                for cc in range(KT):
                                    eng = nc.gpsimd if cc >= 6 else nc.vector
                                    eng.tensor_mul(attsm[:, cc, qsl],
                                                   attsm[:, cc, qsl], bc[:])

                        # (gelu table load happens at the first proj gelu)

                        # ---- proj + gelu (+ residual with head-0 output)
                        _mark(nc, f"h{i}_proj")
                        h_new = hpool.tile([128, KT, S_OWN], F8, tag="hT", name=f"hT{i + 1}")
                        for half in range(2):
                            sl = slice(half * HALF, (half + 1) * HALF)
                            for co in range(KT):
                                ps = psH.tile([128, HALF], F32, tag="mm5")
                                for cc in range(0, KT, 2):
                                    nc.tensor.matmul(
                                        ps[:],
                                        wp_sb[:, cc:cc + 2, co * 128:(co + 1) * 128],
                                        attsm[:, cc:cc + 2, sl],
                                        start=(cc == 0), stop=(cc == KT - 2),
                                        perf_mode=DR)
                                bias_kw = (dict(bias=pb_sb[:, co:co + 1])
                                           if with_bias else {})
                                if i == 0:
                                    nc.scalar.activation(h_new[:, co, sl], ps[:], GELU,
                                                         scale=1.0 / (WS * NORM_SCALE),
                                                         **bias_kw)
                                    nc.vector.tensor_copy(pred[:, co, sl], h_new[:, co, sl])
                                else:
                                    gt = actp.tile([128, HALF], F8, tag="gt")
                                    nc.scalar.activation(gt[:], ps[:], GELU,
                                                         scale=1.0 / (WS * NORM_SCALE),
                                                         **bias_kw)
                                    nc.vector.tensor_add(h_new[:, co, sl], gt[:],
                                                          pred[:, co, sl])
                            if i + 1 == n_heads:
                                # fc_out + pose for this s-half right away
                                if half == 0:
                                    _mark(nc, "fc_out")
                                    if with_bias:
                                        fob_sb = small.tile([1, C], F8, tag="fob")
                                        nc.sync.dma_start(fob_sb[:], fob[:])
                                for ss in range(half * 4, half * 4 + 4):
                                    ps = psA.tile([128, C], F32, tag="mmA")
                                    _mm_full(
                                        nc, ps,
                                        lambda kk, ss=ss: h_new[:, kk:kk + 2, ss * 128:(ss + 1) * 128],
                                        lambda kk, sl2: fow_sb[:, kk:kk + 2, sl2], KT,
                                        extra=(lambda sl2: nc.tensor.matmul(
                                            ps[:, sl2], ones_row8[:], fob_sb[0:1, sl2],
                                            start=False, stop=True)) if with_bias else None)
                                    o_sb = fo2.tile([128, C], F32, tag="osb")
                                    if False:
                                        obf = fo2.tile([128, C], F32, tag="obf")
                                        nc.scalar.activation(obf[:], ps[:], COPY,
                                                             scale=1.0 / WS)
                                        nc.gpsimd.tensor_add(o_sb[:], obf[:],
                                                             pe_sb[:, ss, :])
                                    else:
                                        nc.vector.scalar_tensor_tensor(
                                            o_sb[:], ps[:], 1.0 / WS, pe_sb[:, ss, :],
                                            op0=MULT, op1=mybir.AluOpType.add)
                                    nc.sync.dma_start(out[ss * 128:(ss + 1) * 128, :], o_sb[:])
                        h_own = h_new
                        if i + 1 < n_heads:
                            wq_sb, wk_sb, wv_sb, wp_sb = wq_n, wk_n, wv_n, wp_n


    nc.compile()
    return nc


def build_null() -> bacc.Bacc:
    """Same I/O signature, ~no compute: measures the dispatch floor."""
    nc = bacc.Bacc(num_devices=N_CORES, name="attn_null")
    nc.dram_tensor("x_t", [128, KT, S_OWN], F8, kind="ExternalInput")
    nc.dram_tensor("fc_in_wT", [128, KT, C], F8, kind="ExternalInput")
    nc.dram_tensor("fc_in_b_row", [1, C], F8, kind="ExternalInput")
    nc.dram_tensor("ln_g_row", [1, C], F32, kind="ExternalInput")
    nc.dram_tensor("ln_b_row", [1, C], F32, kind="ExternalInput")
    nc.dram_tensor("wq_t", [H, 128, KT, C], F8, kind="ExternalInput")
    nc.dram_tensor("wk_t", [H, 128, KT, C], F8, kind="ExternalInput")
    nc.dram_tensor("wv_t", [H, 128, KT, C], F8, kind="ExternalInput")
    nc.dram_tensor("wp_t", [H, 128, KT, C], F8, kind="ExternalInput")
    nc.dram_tensor("q_b_col", [H, 128, KT], F32, kind="ExternalInput")
    nc.dram_tensor("k_b_row", [H, 1, C], F8, kind="ExternalInput")
    nc.dram_tensor("v_b_row", [H, 1, C], F8, kind="ExternalInput")
    nc.dram_tensor("proj_b_col", [H, 128, KT], F32, kind="ExternalInput")
    nc.dram_tensor("fc_out_wT", [128, KT, C], F8, kind="ExternalInput")
    nc.dram_tensor("fc_out_b_row", [1, C], F8, kind="ExternalInput")
    nc.dram_tensor("offs", [1, 2], I32, kind="ExternalInput")
    nc.dram_tensor("can_in", [H, 64], F8, kind="ExternalInput")
    pe = nc.dram_tensor("pe", [S_OWN, C], BF16, kind="ExternalInput")
    out = nc.dram_tensor("out", [S_OWN, C], F32, kind="ExternalOutput")
    nc.dram_tensor("canary_out", [H, 64], F8, kind="ExternalOutput")
    with tile.TileContext(nc) as tc:
        with tc.tile_pool(name="p", bufs=2) as p:
            for ss in range(NT_OWN):
                t = p.tile([128, C], BF16, tag="t")
                nc.sync.dma_start(t[:], pe[ss * 128:(ss + 1) * 128, :])
                t2 = p.tile([128, C], F32, tag="t2")
                nc.vector.tensor_copy(t2[:], t[:])
                nc.sync.dma_start(out[ss * 128:(ss + 1) * 128, :], t2[:])
    nc.compile()
    return nc


def _pose_enc_np(s, f):
    pos = np.arange(s, dtype=np.float32)[:, None]
    div = (1.0 / (1000.0 ** (2.0 * np.arange(f, dtype=np.float32) / np.float32(f))))[None, :]
    p = np.zeros((s, f), np.float32)
    p[0::2, :] = np.sin(pos[0::2] * div)
    p[1::2, :] = np.cos(pos[1::2] * div)
    return p


def _f8(a, scale=1.0):
    return np.ascontiguousarray((np.asarray(a, np.float32) * scale).astype(NP8))


def _tile_kt(mat):
    """[C_in, N] -> [128, KT, N] (c_in = kt*128 + partition)."""
    cin, n = mat.shape
    return np.ascontiguousarray(mat.reshape(KT, 128, n).transpose(1, 0, 2))


def prepare_in_maps(x, fc_in_w, fc_in_b, ln_g, ln_b, qkv_w, qkv_b, proj_w, proj_b,
                    fc_out_w, fc_out_b):
    x = np.asarray(x, np.float32)
    qkv_w = np.asarray(qkv_w, np.float32)
    qkv_b = np.asarray(qkv_b, np.float32)
    proj_w = np.asarray(proj_w, np.float32)
    proj_b = np.asarray(proj_b, np.float32)

    shared = {
        "fc_in_wT": _tile_kt(_f8(np.asarray(fc_in_w, np.float32).T, WS)),
        "fc_in_b_row": _f8(np.asarray(fc_in_b)[None, :], WS),
        "ln_g_row": np.ascontiguousarray(np.asarray(ln_g, np.float32)[None, :]),
        "ln_b_row": np.ascontiguousarray(np.asarray(ln_b, np.float32)[None, :]),
        "wq_t": np.stack([_tile_kt(_f8(qkv_w[i, 0:C, :].T, WS)) for i in range(H)]),
        "wk_t": np.stack([_tile_kt(_f8(qkv_w[i, C:2 * C, :].T, WS)) for i in range(H)]),
        "wv_t": np.stack([_tile_kt(_f8(qkv_w[i, 2 * C:, :].T, WS)) for i in range(H)]),
        "wp_t": np.stack([_tile_kt(_f8(proj_w[i].T, WS)) for i in range(H)]),
        "q_b_col": np.ascontiguousarray(
            qkv_b[:, 0:C].reshape(H, KT, 128).transpose(0, 2, 1)),
        "k_b_row": _f8(qkv_b[:, C:2 * C][:, None, :], WS),
        "v_b_row": _f8(qkv_b[:, 2 * C:][:, None, :], WS),
        "proj_b_col": np.ascontiguousarray(
            proj_b.reshape(H, KT, 128).transpose(0, 2, 1)),
        "fc_out_wT": _tile_kt(_f8(np.asarray(fc_out_w, np.float32).T, WS)),
        "fc_out_b_row": _f8(np.asarray(fc_out_b)[None, :], WS),
    }
    pe_full = _pose_enc_np(S, C)
    in_maps = []
    for core in range(N_CORES):
        b, half = divmod(core, 2)
        own = x[b, half * S_OWN:(half + 1) * S_OWN, :].T  # [C, S_OWN]
        m = dict(shared)
        m["x_t"] = _tile_kt(_f8(own))
        m["pe"] = np.ascontiguousarray(
            pe_full[half * S_OWN:(half + 1) * S_OWN, :].astype(NPBF))
        slot_elems = 128 * (KT * C + 64)
        m["offs"] = np.array([[half * slot_elems, (1 - half) * slot_elems]], np.int32)
        m["can_in"] = np.stack(
            [np.full((64,), float((core + 1) * (2 ** i)), NP8) for i in range(H)])
        in_maps.append(m)
    return in_maps


_NC_CACHE = {}


def get_nc(n_heads=H, with_bias=True, with_ln_affine=True):
    key = (n_heads, with_bias, with_ln_affine)
    if key not in _NC_CACHE:
        _NC_CACHE[key] = build(n_heads, with_bias, with_ln_affine)
    return _NC_CACHE[key]


_EXEC_CACHE = {}


def _get_executable(nc):
    """One jitted collectives executable per process (loading a second one
    hangs the axon worker); reused across kernel() calls."""
    key = id(nc)
    if key in _EXEC_CACHE:
        return _EXEC_CACHE[key]
    import jax
    from jax.sharding import Mesh, PartitionSpec, NamedSharding
    from jax.experimental.shard_map import shard_map
    from concourse import bass2jax
    import concourse.mybir as mybir_

    bass2jax.install_neuronx_cc_hook()
    partition_name = nc.partition_id_tensor.name if nc.partition_id_tensor else None
    in_names, out_names, out_avals, zero_outs = [], [], [], []
    for alloc in nc.m.functions[0].allocations:
        if not isinstance(alloc, mybir_.MemoryLocationSet):
            continue
        name = alloc.memorylocations[0].name
        if alloc.kind == "ExternalInput":
            if name != partition_name:
                in_names.append(name)
        elif alloc.kind == "ExternalOutput":
            out_names.append(name)
            shape = tuple(alloc.tensor_shape)
            dtype = mybir_.dt.np(alloc.dtype)
            out_avals.append(jax.core.ShapedArray(shape, dtype))
            zero_outs.append(np.zeros(shape, dtype))
    n_params = len(in_names)
    n_outs = len(out_avals)
    all_in = in_names + out_names + ([partition_name] if partition_name else [])
    donate = tuple(range(n_params, n_params + n_outs))

    def _body(*args):
        operands = list(args)
        if partition_name is not None:
            operands.append(bass2jax.partition_id_tensor())
        return tuple(bass2jax._bass_exec_p.bind(
            *operands, out_avals=tuple(out_avals), in_names=tuple(all_in),
            out_names=tuple(out_names), lowering_input_output_aliases=(),
            sim_require_finite=True, sim_require_nnan=True, nc=nc))

    devices = jax.devices()[:N_CORES]
    mesh = Mesh(np.asarray(devices), ("core",))
    sharded = jax.jit(
        shard_map(_body, mesh=mesh,
                  in_specs=(PartitionSpec("core"),) * (n_params + n_outs),
                  out_specs=(PartitionSpec("core"),) * len(out_names),
                  check_rep=False),
        donate_argnums=donate, keep_unused=True)
    sh = NamedSharding(mesh, PartitionSpec("core"))
    entry = (sharded, sh, in_names[:n_params], out_names, out_avals, zero_outs)
    _EXEC_CACHE[key] = entry
    return entry


def flags_for(inputs):
    with_bias = not (np.all(np.asarray(inputs["fc_in_b"]) == 0)
                     and np.all(np.asarray(inputs["qkv_b"]) == 0)
                     and np.all(np.asarray(inputs["proj_b"]) == 0)
                     and np.all(np.asarray(inputs["fc_out_b"]) == 0))
    with_ln = not (np.all(np.asarray(inputs["ln_g"]) == 1)
                   and np.all(np.asarray(inputs["ln_b"]) == 0))
    return with_bias, with_ln


_WARMED = set()


def canaries_ok(out_arrs, out_names, out_avals):
    """True iff every core read its partner's per-head canary in every head:
    proves each head's exchange (including the early-barrier race) was clean
    for that execution."""
    ci = out_names.index("canary_out")
    pc = np.asarray(out_arrs[ci]).reshape(N_CORES, *out_avals[ci].shape)
    vals = pc.astype(np.float32)
    for core in range(N_CORES):
        for i in range(vals.shape[1]):
            if not np.all(vals[core, i] == float(((core ^ 1) + 1) * (2 ** i))):
                return False
    return True


def kernel(**inputs) -> np.ndarray:
    with_bias, with_ln = flags_for(inputs)
    nc = get_nc(H, with_bias, with_ln)
    in_maps = prepare_in_maps(**inputs)
    import jax
    sharded, sh, in_names, out_names, out_avals, zero_outs = _get_executable(nc)
    concat_in = [jax.device_put(
        np.concatenate([np.asarray(in_maps[c][nm]) for c in range(N_CORES)], axis=0), sh)
        for nm in in_names]

    def one_call():
        concat_zeros = [jax.device_put(
            np.zeros((N_CORES * z.shape[0], *z.shape[1:]), z.dtype), sh)
            for z in zero_outs]
        out_arrs = sharded(*concat_in, *concat_zeros)
        jax.block_until_ready(out_arrs)
        return out_arrs

    # The first executions after NEFF load race DGE descriptor generation
    # against the dynamic-offset register loads; registers persist across
    # executions, so retry until the canaries prove the exchange addressed
    # the right slots (typically clean from the 2nd execution).
    if id(nc) not in _WARMED:
        one_call()
        _WARMED.add(id(nc))
    for _attempt in range(8):
        out_arrs = one_call()
        oi_ = out_names.index("out")
        pc_ = np.asarray(out_arrs[oi_])
        if canaries_ok(out_arrs, out_names, out_avals) and not np.isnan(pc_).any():
            break
    oi = out_names.index("out")
    per_core = np.asarray(out_arrs[oi]).reshape(N_CORES, *out_avals[oi].shape)
    out_full = np.empty((B, S, C), np.float32)
    for core in range(N_CORES):
        b, half = divmod(core, 2)
        out_full[b, half * S_OWN:(half + 1) * S_OWN, :] = per_core[core]
    return out_full


# revision 8
# speedup vs baseline: 1.3649x; 1.0016x over previous
"""Distributed Trainium2 kernel for nn_Attention_18562848653411 (v2).

Reference model: fc_in -> LayerNorm -> 4 sequential "refinement heads"
(qkv matmul + gelu, scores=q@k^T/C, att=scores@v, softmax over channels,
proj + gelu, residual with head-0 output) -> fc_out + PoseEncoding.

Key algebra: softmax comes AFTER att = scores@v, so per head
att^T = (v^T k) q^T / C = M q^T / C with M = v^T k a [C,C] matrix that
is a sum over sequence positions. No S x S scores are ever formed.

Sharding (8 NeuronCores): core c handles batch b=c//2, sequence half
c%2. All weights replicated, fp8e4 (x16 host scale); every big matmul is
a DoubleRow fp8 instruction. Per head each core computes k/v and
M_own = v_own^T k_own over its OWN 1024 rows and exchanges M_own (1 MB
fp8) with its pair partner through pair-shared HBM. The pair barrier
(tiny AllGather) is issued EARLY (right after k/v) so its ~15us fixed
latency overlaps the M matmuls and M writes; per-head canary values
(core+1)*2^head written after the M payload on the same queue witness
that the partner's writes landed before our read -- kernel() retries
until canaries prove a clean exchange.

Softmax over channels runs on transposed tiles att^T[c, s]: exp via
activation (scale=1/16, bias=-8ln2), DoubleRow ones-matmul denominator,
gpsimd partition-broadcast of 256/denom, DVE normalize; proj descales by
1/(16*256) inside its gelu. The apply step accumulates m_own q and
m_partner q directly in PSUM (no pre-add of the M halves). LayerNorm
runs in row space off the fc_in PSUM (bn_stats, Act does only Sqrt so a
single act table serves all of fc_in); h^T comes from PE transposes +
Act Copy (no DMA transposes).
"""

import numpy as np
import ml_dtypes

import concourse.bass as bass
import concourse.mybir as mybir
import concourse.tile as tile
from concourse import bacc
from concourse.bass_utils import run_bass_kernel_spmd  # noqa: F401
from concourse.masks import make_identity

N_CORES = 8
PAIRS = [[0, 1], [2, 3], [4, 5], [6, 7]]
B, S, C = 4, 2048, 1024
H = 4
S_OWN = S // 2
KT = C // 128          # 8 contraction tiles of 128
NT_OWN = S_OWN // 128  # 8 own t tiles
HALF = 512

F32 = mybir.dt.float32
BF16 = mybir.dt.bfloat16
F8 = mybir.dt.float8e4
I32 = mybir.dt.int32
GELU = mybir.ActivationFunctionType.Gelu
EXP = mybir.ActivationFunctionType.Exp
SQRT = mybir.ActivationFunctionType.Sqrt
COPY = mybir.ActivationFunctionType.Copy
SUB = mybir.AluOpType.subtract
MULT = mybir.AluOpType.mult
BYPASS = mybir.AluOpType.bypass
DR = mybir.MatmulPerfMode.DoubleRow

NP8 = ml_dtypes.float8_e4m3fn
NPBF = ml_dtypes.bfloat16

WS = 16.0             # host weight scale
MSC = 64.0            # M stored as M/MSC
APS = C / MSC         # apply psum = APS * att_raw = 16*att
EXP_SHIFT = 8.0       # exp output scaled 2^-8
NORM_SCALE = 256.0    # normalized att stored x256
M_ELEMS = 128 * KT * C  # one M half (1 MB fp8)
GATE_TT = 1  # v tile whose completion launches the pair barrier

PHASE_MARKS = []


def _mark(nc, name):
    PHASE_MARKS.append((name, int(nc.get_next_instruction_name().split("-")[1])))


def _mm_full(nc, ps, lhsT_of, rhs_of, n_k, extra=None):
    """Accumulate a [128, 1024] psum tile in two 512-col bank halves with
    DoubleRow fp8 matmuls. lhsT_of(kk) -> [128,2,128]; rhs_of(kk, sl) ->
    [128,2,512]. extra(sl) appends a bias matmul closing the group."""
    for half in range(2):
        sl = slice(half * HALF, (half + 1) * HALF)
        for kk in range(0, n_k, 2):
            nc.tensor.matmul(ps[:, sl], lhsT_of(kk), rhs_of(kk, sl),
                             start=(kk == 0),
                             stop=(extra is None and kk == n_k - 2),
                             perf_mode=DR)
        if extra is not None:
            extra(sl)


def build(n_heads: int = H, with_bias: bool = True, with_ln_affine: bool = True) -> bacc.Bacc:
    PHASE_MARKS.clear()
    nc = bacc.Bacc(num_devices=N_CORES, name="attn")

    x_t = nc.dram_tensor("x_t", [128, KT, S_OWN], F8, kind="ExternalInput")
    fcw = nc.dram_tensor("fc_in_wT", [128, KT, C], F8, kind="ExternalInput")
    fcb = nc.dram_tensor("fc_in_b_row", [1, C], F8, kind="ExternalInput")
    lng = nc.dram_tensor("ln_g_row", [1, C], F32, kind="ExternalInput")
    lnb = nc.dram_tensor("ln_b_row", [1, C], F32, kind="ExternalInput")
    wq = nc.dram_tensor("wq_t", [H, 128, KT, C], F8, kind="ExternalInput")
    wk = nc.dram_tensor("wk_t", [H, 128, KT, C], F8, kind="ExternalInput")
    wv = nc.dram_tensor("wv_t", [H, 128, KT, C], F8, kind="ExternalInput")
    wp = nc.dram_tensor("wp_t", [H, 128, KT, C], F8, kind="ExternalInput")
    qb = nc.dram_tensor("q_b_col", [H, 128, KT], F32, kind="ExternalInput")
    kb = nc.dram_tensor("k_b_row", [H, 1, C], F8, kind="ExternalInput")
    vb = nc.dram_tensor("v_b_row", [H, 1, C], F8, kind="ExternalInput")
    pb = nc.dram_tensor("proj_b_col", [H, 128, KT], F32, kind="ExternalInput")
    fow = nc.dram_tensor("fc_out_wT", [128, KT, C], F8, kind="ExternalInput")
    fob = nc.dram_tensor("fc_out_b_row", [1, C], F8, kind="ExternalInput")
    offs = nc.dram_tensor("offs", [1, 2], I32, kind="ExternalInput")
    can_in = nc.dram_tensor("can_in", [H, 64], F8, kind="ExternalInput")
    pe = nc.dram_tensor("pe", [S_OWN, C], BF16, kind="ExternalInput")
    out = nc.dram_tensor("out", [S_OWN, C], F32, kind="ExternalOutput")
    canary_out = nc.dram_tensor("canary_out", [H, 64], F8, kind="ExternalOutput")

    ROWE = KT * C + 64  # row stride: M payload + canary pad
    hsh = [nc.dram_tensor(f"hsh{i}", [2, 128, ROWE], F8,
                          kind="Internal", addr_space="Shared")
           for i in range(n_heads)]
    bar_in = nc.dram_tensor("bar_in", [1, 3], F8, kind="Internal")
    bar_out = [nc.dram_tensor(f"bar_out{i}", [2, 3], F8, kind="Internal")
               for i in range(n_heads)]

    with tile.TileContext(nc) as tc:
        with (
            tc.tile_pool(name="pers", bufs=1) as pers,
            tc.tile_pool(name="hpool", bufs=2) as hpool,
            tc.tile_pool(name="wpool", bufs=2) as wpool,
            tc.tile_pool(name="small", bufs=2) as small,
            tc.tile_pool(name="act", bufs=2) as actp,
        ):
            ones16 = pers.tile([128, 2, 16], F8)
            nc.vector.memset(ones16[:], 1.0)
            if with_bias:
                ones_row8 = pers.tile([1, 128], F8)
                nc.vector.memset(ones_row8[:], 1.0)
            eps_t = pers.tile([128, 1], F32)
            nc.vector.memset(eps_t[:], 1e-5)
            expb = pers.tile([128, 1], F32)
            nc.vector.memset(expb[:], -float(EXP_SHIFT) * float(np.log(2.0)))
            ident = pers.tile([128, 128], BF16)
            make_identity(nc, ident[:])
            pred = pers.tile([128, KT, S_OWN], F8, name="pred")
            pe_sb = pers.tile([128, NT_OWN, C], BF16, name="pe_sb")

            off_sb = pers.tile([1, 2], I32)
            nc.sync.dma_start(off_sb[:], offs[:])
            r_w = nc.sync.alloc_register("r_w")
            r_r = nc.sync.alloc_register("r_r")
            nc.sync.reg_load(r_w, off_sb[0:1, 0:1])
            nc.sync.reg_load(r_r, off_sb[0:1, 1:2])
            r_wc = nc.sync.alloc_register("r_wc")
            r_rc = nc.sync.alloc_register("r_rc")
            r_w2 = nc.sync.alloc_register("r_w2")
            r_r2 = nc.sync.alloc_register("r_r2")
            nc.sync.reg_add(r_wc, r_w, KT * C)
            nc.sync.reg_add(r_rc, r_r, KT * C)
            nc.sync.reg_add(r_w2, r_w, KT * C // 2)
            nc.sync.reg_add(r_r2, r_r, KT * C // 2)
            can_sb = pers.tile([H, 64], F8)
            nc.sync.dma_start(can_sb[:], can_in[:])

            if with_bias:
                kb_sb = small.tile([1, C], F8, tag="kb")
                nc.sync.dma_start(kb_sb[:], kb[0])
                vb_sb = small.tile([1, C], F8, tag="vb")
                nc.sync.dma_start(vb_sb[:], vb[0])
            k0_sb = None  # head-0 k/v tiles, filled by the fc_in loop

            # ================= fc_in + LayerNorm (own rows only) ============
            _mark(nc, "fc_in")
            h_own = hpool.tile([128, KT, S_OWN], F8, tag="hT", name="hT0")
            k0_sb = pers.tile([128, NT_OWN, C], F8, name="k0_sb")
            v0_sb = pers.tile([128, NT_OWN, C], F8, name="v0_sb")
            with (tc.tile_pool(name="s0", bufs=1) as s0,
                  tc.tile_pool(name="s0ps", bufs=3, space="PSUM") as s0ps,
                  tc.tile_pool(name="s0tp", bufs=2, space="PSUM") as s0tp):
                fcw_sb = s0.tile([128, KT, C], F8)
                nc.sync.dma_start(fcw_sb[:], fcw[:])
                x_sb = s0.tile([128, KT, S_OWN], F8)
                nc.sync.dma_start(x_sb[:, :, 0:HALF], x_t[:, :, 0:HALF])
                nc.sync.dma_start(x_sb[:, :, HALF:S_OWN], x_t[:, :, HALF:S_OWN])
                # head-0 weights prefetch (gpsimd/SWDGE) after the fc_in
                # inputs so they don't delay the first matmul; wk/wv first
                # (the fc_in loop tail computes head-0 k/v)
                wk_sb = wpool.tile([128, KT, C], F8, tag="wk")
                nc.gpsimd.dma_start(wk_sb[:], wk[0])
                wv_sb = wpool.tile([128, KT, C], F8, tag="wv")
                nc.gpsimd.dma_start(wv_sb[:], wv[0])
                wq_sb = wpool.tile([128, KT, C], F8, tag="wq")
                nc.gpsimd.dma_start(wq_sb[:], wq[0])
                wp_sb = wpool.tile([128, KT, C], F8, tag="wp")
                nc.gpsimd.dma_start(wp_sb[:], wp[0])
                if with_bias:
                    fcb_sb = s0.tile([1, C], F8)
                    nc.sync.dma_start(fcb_sb[:], fcb[:])
                if with_ln_affine:
                    g_bc = s0.tile([128, C], F32)
                    nc.sync.dma_start(g_bc[:], bass.AP(tensor=lng, offset=0,
                                                       ap=[[0, 128], [1, C]]))
                    b_bc = s0.tile([128, C], F32)
                    nc.sync.dma_start(b_bc[:], bass.AP(tensor=lnb, offset=0,
                                                       ap=[[0, 128], [1, C]]))
                for blk in range(1):
                    # 4 fc_in tiles (Act: sqrt+copy, one table); the PE
                    # transposes trail the mm/LN chain by one tile so the next
                    # matmul never waits on the DVE normalize
                    hnb_q = []

                    def _transp(ss, hnb):
                        for grp in range(2):
                            tp = s0tp.tile([128, 4, 128], BF16, tag="tp")
                            for j in range(4):
                                cc = grp * 4 + j
                                nc.tensor.transpose(
                                    tp[:, j, :], hnb[:, cc * 128:(cc + 1) * 128], ident[:])
                            nc.scalar.activation(
                                h_own[:, grp * 4:(grp + 1) * 4, ss * 128:(ss + 1) * 128],
                                tp[:], COPY)

                    for ss in range(8):
                        ps = s0ps.tile([128, C], F32, tag="mmA")
                        _mm_full(
                            nc, ps,
                            lambda kk, ss=ss: x_sb[:, kk:kk + 2, ss * 128:(ss + 1) * 128],
                            lambda kk, sl: fcw_sb[:, kk:kk + 2, sl], KT,
                            extra=(lambda sl: nc.tensor.matmul(
                                ps[:, sl], ones_row8[:], fcb_sb[0:1, sl],
                                start=False, stop=True)) if with_bias else None)
                        stats = small.tile([128, 2, 6], F32, tag="bnst")
                        nc.vector.bn_stats(stats[:, 0, :], ps[:, 0:HALF])
                        nc.vector.bn_stats(stats[:, 1, :], ps[:, HALF:C])
                        mv = small.tile([128, 2], F32, tag="mv")
                        nc.vector.bn_aggr(mv[:], stats[:])
                        rstd = small.tile([128, 1], F32, tag="rstd")
                        nc.scalar.activation(rstd[:], mv[:, 1:2], SQRT, bias=eps_t[:], scale=1.0)
                        nc.vector.reciprocal(rstd[:], rstd[:])
                        # hnb = ps*rstd - mu*rstd: scale on Act (Copy works in
                        # any act table), subtract on DVE in 4x bf16 mode
                        murs = small.tile([128, 1], F32, tag="murs")
                        nc.vector.tensor_mul(murs[:], mv[:, 0:1], rstd[:])
                        hsc = s0.tile([128, C], BF16, tag="hsc", bufs=2)
                        nc.scalar.activation(hsc[:], ps[:], COPY, scale=rstd[:])
                        hnb = s0.tile([128, C], BF16, tag="hnb", bufs=3)
                        nc.vector.tensor_scalar_sub(hnb[:], hsc[:], murs[:])
                        if with_ln_affine:
                            hnf = s0.tile([128, C], F32, tag="hnf", bufs=2)
                            nc.vector.tensor_mul(hnf[:], hnb[:], g_bc[:])
                            nc.vector.tensor_add(hnb[:], hnf[:], b_bc[:])
                        hnb_q.append((ss, hnb))
                        if len(hnb_q) > 1:
                            _transp(*hnb_q.pop(0))
                    while hnb_q:
                        _transp(*hnb_q.pop(0))
                    # ... then head-0 k/v for those tiles (Act: gelu) -- block
                    # granularity keeps act-table transitions to one per block
                    for tt in range(8):
                        for dst, wmat, bias_sb in ((k0_sb, wk_sb, kb_sb if with_bias else None),
                                                   (v0_sb, wv_sb, vb_sb if with_bias else None)):
                            ps = s0ps.tile([128, C], F32, tag="mmA")
                            _mm_full(
                                nc, ps,
                                lambda kk, tt=tt: h_own[:, kk:kk + 2, tt * 128:(tt + 1) * 128],
                                lambda kk, sl, wmat=wmat: wmat[:, kk:kk + 2, sl], KT,
                                extra=(lambda sl, b=bias_sb: nc.tensor.matmul(
                                    ps[:, sl], ones_row8[:], b[0:1, sl],
                                    start=False, stop=True)) if with_bias else None)
                            nc.scalar.activation(dst[:, tt, :], ps[:], GELU, scale=1.0 / WS)

            # (gelu table load happens at head 0's first kv gelu)

            # ================= heads =================
            with (tc.tile_pool(name="psA", bufs=2, space="PSUM") as psA,
                  tc.tile_pool(name="psH", bufs=3, space="PSUM") as psH,
                  tc.tile_pool(name="psD", bufs=1, space="PSUM") as psD,
                  tc.tile_pool(name="fo2", bufs=2) as fo2):
                for i in range(n_heads):
                    wi = i % H
                    _mark(nc, f"head{i}")
                    with tc.tile_pool(name=f"hd{i}", bufs=1) as hp:
                        if with_bias:
                            if i > 0:
                                kb_sb = small.tile([1, C], F8, tag="kb")
                                nc.sync.dma_start(kb_sb[:], kb[wi])
                                vb_sb = small.tile([1, C], F8, tag="vb")
                                nc.sync.dma_start(vb_sb[:], vb[wi])
                            qb_sb = small.tile([128, KT], F32, tag="qb")
                            nc.sync.dma_start(qb_sb[:], qb[wi])
                            pb_sb = small.tile([128, KT], F32, tag="pb")
                            nc.sync.dma_start(pb_sb[:], pb[wi])

                        q_sb = hp.tile([128, KT, S_OWN], F8, name="q_sb")
                        if i == 0:
                            k_sb, v_sb = k0_sb, v0_sb
                        else:
                            k_sb = hp.tile([128, NT_OWN, C], F8, name="k_sb")
                            v_sb = hp.tile([128, NT_OWN, C], F8, name="v_sb")
                        m_sb = hp.tile([128, KT, C], F8, name="m_sb")
                        mp_sb = hp.tile([128, KT, C], F8, name="mp_sb")
                        attsm = hp.tile([128, KT, S_OWN], F8, name="attsm")

                        # ---- k, v [t, c] (h-stationary) over own rows
                        # (head 0's k/v were interleaved into the fc_in loop)
                        if i > 0:
                            kv_list = ((k_sb, wk_sb, kb_sb if with_bias else None),
                                       (v_sb, wv_sb, vb_sb if with_bias else None))
                            for dst, wmat, bias_sb in kv_list:
                                for tt in range(NT_OWN):
                                    ps = psA.tile([128, C], F32, tag="mmA")
                                    _mm_full(
                                        nc, ps,
                                        lambda kk, tt=tt: h_own[:, kk:kk + 2, tt * 128:(tt + 1) * 128],
                                        lambda kk, sl, wmat=wmat: wmat[:, kk:kk + 2, sl], KT,
                                        extra=(lambda sl, b=bias_sb: nc.tensor.matmul(
                                            ps[:, sl], ones_row8[:], b[0:1, sl],
                                            start=False, stop=True)) if with_bias else None)
                                    nc.scalar.activation(dst[:, tt, :], ps[:], GELU, scale=1.0 / WS)

                        # ---- early pair barrier: gate bar_in on v's last tile
                        # so the ~15us collective overlaps the M matmuls and
                        # M writes (canaries verify the race was won).
                        _mark(nc, f"h{i}_bar")
                        # barrier payload is irrelevant; source it from v tile 3
                        # so the collective launches once v is half done (the
                        # remaining ~15us of barrier covers M compute + writes
                        # on both cores; canaries verify the race was won)
                        nc.sync.dma_start(bar_in[:], v_sb[0:1, GATE_TT, 0:3])
                        nc.gpsimd.collective_compute(
                            "AllGather", BYPASS, replica_groups=PAIRS,
                            ins=[bar_in[:].opt()], outs=[bar_out[i][:].opt()])

                        # next-head weight prefetch (gpsimd queue, after the
                        # collective so transfers run under the barrier)
                        if i + 1 < n_heads:
                            nwi = (i + 1) % H
                            wq_n = wpool.tile([128, KT, C], F8, tag="wq")
                            nc.gpsimd.dma_start(wq_n[:], wq[nwi])
                            wk_n = wpool.tile([128, KT, C], F8, tag="wk")
                            nc.gpsimd.dma_start(wk_n[:], wk[nwi])
                            wv_n = wpool.tile([128, KT, C], F8, tag="wv")
                            nc.gpsimd.dma_start(wv_n[:], wv[nwi])
                            wp_n = wpool.tile([128, KT, C], F8, tag="wp")
                            nc.gpsimd.dma_start(wp_n[:], wp[nwi])
                        if i == 1:
                            nc.gpsimd.dma_start(
                                pe_sb[:], bass.AP(tensor=pe, offset=0,
                                                  ap=[[C, 128], [128 * C, NT_OWN], [1, C]]))
                        if i == 2:
                            fow_sb = pers.tile([128, KT, C], F8, name="fow_sb")
                            nc.gpsimd.dma_start(fow_sb[:], fow[:])

                        # ---- M_own = v_own^T k_own (x 1/MSC), [c, cq]
                        _mark(nc, f"h{i}_M")
                        for co in range(KT):
                            ps = psA.tile([128, C], F32, tag="mmA")
                            _mm_full(
                                nc, ps,
                                lambda tt, co=co: v_sb[:, tt:tt + 2, co * 128:(co + 1) * 128],
                                lambda tt, sl: k_sb[:, tt:tt + 2, sl], NT_OWN)
                            nc.vector.tensor_scalar_mul(m_sb[:, co, :], ps[:], 1.0 / MSC)
                            if co == KT // 2 - 1:
                                wap = bass.AP(tensor=hsh[i], offset=r_w,
                                              ap=[[ROWE, 128], [1, KT * C // 2]],
                                              dep_tracking_offset=0)
                                nc.sync.dma_start(wap, m_sb[:, 0:KT // 2, :])
                        wap2 = bass.AP(tensor=hsh[i], offset=r_w2,
                                       ap=[[ROWE, 128], [1, KT * C // 2]],
                                       dep_tracking_offset=KT * C // 2)
                        nc.sync.dma_start(wap2, m_sb[:, KT // 2:KT, :])
                        wcap = bass.AP(tensor=hsh[i], offset=r_wc, ap=[[64, 1], [1, 64]],
                                       dep_tracking_offset=KT * C)
                        nc.sync.dma_start(wcap, can_sb[wi:wi + 1, :])

                        # ---- q [co, s] (w-stationary) -- fills barrier window
                        _mark(nc, f"h{i}_q")
                        for co in range(KT):
                            ps = psA.tile([128, S_OWN], F32, tag="mmA")
                            _mm_full(
                                nc, ps,
                                lambda kk, co=co: wq_sb[:, kk:kk + 2, co * 128:(co + 1) * 128],
                                lambda kk, sl: h_own[:, kk:kk + 2, sl], KT)
                            if with_bias:
                                nc.scalar.activation(q_sb[:, co, :], ps[:], GELU,
                                                     bias=qb_sb[:, co:co + 1], scale=1.0 / WS)
                            else:
                                nc.scalar.activation(q_sb[:, co, :], ps[:], GELU,
                                                     scale=1.0 / WS)

                        # (exp table load happens at the first apply exp)

                        # ---- barrier done: canary first, then partner M
                        bar_sb = hp.tile([2, 3], F8, name="bar_sb")
                        nc.sync.dma_start(bar_sb[:], bar_out[i][:])
                        rcap = bass.AP(tensor=hsh[i], offset=r_rc, ap=[[64, 1], [1, 64]],
                                       dep_tracking_offset=M_ELEMS + KT * C)
                        can_rd = hp.tile([1, 64], F8, name="can_rd")
                        nc.sync.dma_start(can_rd[:], rcap)
                        nc.sync.dma_start(canary_out[wi:wi + 1, :], can_rd[:])
                        rap0 = bass.AP(tensor=hsh[i], offset=r_r,
                                       ap=[[ROWE, 128], [1, KT * C // 2]],
                                       dep_tracking_offset=M_ELEMS)
                        nc.sync.dma_start(mp_sb[:, 0:KT // 2, :], rap0)
                        rap1 = bass.AP(tensor=hsh[i], offset=r_r2,
                                       ap=[[ROWE, 128], [1, KT * C // 2]],
                                       dep_tracking_offset=M_ELEMS + KT * C // 2)
                        nc.sync.dma_start(mp_sb[:, KT // 2:KT, :], rap1)

                        # ---- apply + exp, per s-half. Half 0 accumulates m
                        # and mp straight in PSUM (starts when the partner
                        # chunk lands); half 1 uses the DVE pre-added msum
                        # (computed in half 0's shadow), halving its matmuls.
                        _mark(nc, f"h{i}_apply")
                        msum = hp.tile([128, KT, C], F8, name="msum")
                        for half in range(2):
                            sl = slice(half * HALF, (half + 1) * HALF)
                            for co in range(KT):
                                ps = psH.tile([128, HALF], F32, tag="mm5")
                                if half == 0:
                                    for src_i, msrc in enumerate((m_sb, mp_sb)):
                                        for cc in range(0, KT, 2):
                                            nc.tensor.matmul(
                                                ps[:],
                                                msrc[:, cc:cc + 2, co * 128:(co + 1) * 128],
                                                q_sb[:, cc:cc + 2, sl],
                                                start=(src_i == 0 and cc == 0),
                                                stop=(src_i == 1 and cc == KT - 2),
                                                perf_mode=DR)
                                else:
                                    for cc in range(0, KT, 2):
                                        nc.tensor.matmul(
                                            ps[:],
                                            msum[:, cc:cc + 2, co * 128:(co + 1) * 128],
                                            q_sb[:, cc:cc + 2, sl],
                                            start=(cc == 0), stop=(cc == KT - 2),
                                            perf_mode=DR)
                                nc.scalar.activation(attsm[:, co, sl], ps[:], EXP,
                                                     bias=expb[:], scale=1.0 / APS)
                            if half == 0:
                                # msum for half 1, under half 0's matmuls
                                for cc in range(KT):
                                    nc.vector.tensor_add(msum[:, cc, :], m_sb[:, cc, :],
                                                         mp_sb[:, cc, :])
                            # denominator (DoubleRow ones) + 256/denom
                            # broadcast, pipelined per s-quarter so proj can
                            # chase the normalize front
                            for qq in range(2):
                                qsl = slice(half * HALF + qq * 256,
                                            half * HALF + (qq + 1) * 256)
                                dn = psD.tile([16, 256], F32, tag="dn")
                                for cc in range(0, KT, 2):
                                    nc.tensor.matmul(dn[:], ones16[:],
                                                     attsm[:, cc:cc + 2, qsl],
                                                     start=(cc == 0), stop=(cc == KT - 2),
                                                     perf_mode=DR)
                                rr = small.tile([1, 256], F32, tag="rr")
                                nc.vector.reciprocal(rr[:], dn[0:1, :])
                                rr2 = small.tile([1, 256], F32, tag="rr2")
                                nc.vector.tensor_scalar_mul(rr2[:], rr[:], NORM_SCALE)
                                bc = small.tile([128, 256], F32, tag="bc")
                                nc.gpsimd.partition_broadcast(bc[:], rr2[0:1, :], channels=128)
                                for cc in range(KT):
                                    eng = nc.gpsimd if cc >= 6 else nc.vector
                                    eng.tensor_mul(attsm[:, cc, qsl],
                                                   attsm[:, cc, qsl], bc[:])

                        # (gelu table load happens at the first proj gelu)

                        # ---- proj + gelu (+ residual with head-0 output)
                        _mark(nc, f"h{i}_proj")
                        h_new = hpool.tile([128, KT, S_OWN], F8, tag="hT", name=f"hT{i + 1}")
                        for half in range(2):
                            sl = slice(half * HALF, (half + 1) * HALF)
                            for co in range(KT):
                                ps = psH.tile([128, HALF], F32, tag="mm5")
                                for cc in range(0, KT, 2):
                                    nc.tensor.matmul(
                                        ps[:],
                                        wp_sb[:, cc:cc + 2, co * 128:(co + 1) * 128],
                                        attsm[:, cc:cc + 2, sl],
                                        start=(cc == 0), stop=(cc == KT - 2),
                                        perf_mode=DR)
                                bias_kw = (dict(bias=pb_sb[:, co:co + 1])
                                           if with_bias else {})
                                if i == 0:
                                    nc.scalar.activation(h_new[:, co, sl], ps[:], GELU,
                                                         scale=1.0 / (WS * NORM_SCALE),
                                                         **bias_kw)
                                    nc.vector.tensor_copy(pred[:, co, sl], h_new[:, co, sl])
                                else:
                                    gt = actp.tile([128, HALF], F8, tag="gt")
                                    nc.scalar.activation(gt[:], ps[:], GELU,
                                                         scale=1.0 / (WS * NORM_SCALE),
                                                         **bias_kw)
                                    nc.vector.tensor_add(h_new[:, co, sl], gt[:],
                                                          pred[:, co, sl])
                            if i + 1 == n_heads:
                                # fc_out + pose for this s-half right away
                                if half == 0:
                                    _mark(nc, "fc_out")
                                    if with_bias:
                                        fob_sb = small.tile([1, C], F8, tag="fob")
                                        nc.sync.dma_start(fob_sb[:], fob[:])
                                for ss in range(half * 4, half * 4 + 4):
                                    ps = psA.tile([128, C], F32, tag="mmA")
                                    _mm_full(
                                        nc, ps,
                                        lambda kk, ss=ss: h_new[:, kk:kk + 2, ss * 128:(ss + 1) * 128],
                                        lambda kk, sl2: fow_sb[:, kk:kk + 2, sl2], KT,
                                        extra=(lambda sl2: nc.tensor.matmul(
                                            ps[:, sl2], ones_row8[:], fob_sb[0:1, sl2],
                                            start=False, stop=True)) if with_bias else None)
                                    o_sb = fo2.tile([128, C], F32, tag="osb")
                                    if False:
                                        obf = fo2.tile([128, C], F32, tag="obf")
                                        nc.scalar.activation(obf[:], ps[:], COPY,
                                                             scale=1.0 / WS)
                                        nc.gpsimd.tensor_add(o_sb[:], obf[:],
                                                             pe_sb[:, ss, :])
                                    else:
                                        nc.vector.scalar_tensor_tensor(
                                            o_sb[:], ps[:], 1.0 / WS, pe_sb[:, ss, :],
                                            op0=MULT, op1=mybir.AluOpType.add)
                                    nc.sync.dma_start(out[ss * 128:(ss + 1) * 128, :], o_sb[:])
                        h_own = h_new
                        if i + 1 < n_heads:
                            wq_sb, wk_sb, wv_sb, wp_sb = wq_n, wk_n, wv_n, wp_n


    nc.compile()
    return nc


def build_null() -> bacc.Bacc:
    """Same I/O signature, ~no compute: measures the dispatch floor."""
    nc = bacc.Bacc(num_devices=N_CORES, name="attn_null")
    nc.dram_tensor("x_t", [128, KT, S_OWN], F8, kind="ExternalInput")
    nc.dram_tensor("fc_in_wT", [128, KT, C], F8, kind="ExternalInput")
    nc.dram_tensor("fc_in_b_row", [1, C], F8, kind="ExternalInput")
    nc.dram_tensor("ln_g_row", [1, C], F32, kind="ExternalInput")
    nc.dram_tensor("ln_b_row", [1, C], F32, kind="ExternalInput")
    nc.dram_tensor("wq_t", [H, 128, KT, C], F8, kind="ExternalInput")
    nc.dram_tensor("wk_t", [H, 128, KT, C], F8, kind="ExternalInput")
    nc.dram_tensor("wv_t", [H, 128, KT, C], F8, kind="ExternalInput")
    nc.dram_tensor("wp_t", [H, 128, KT, C], F8, kind="ExternalInput")
    nc.dram_tensor("q_b_col", [H, 128, KT], F32, kind="ExternalInput")
    nc.dram_tensor("k_b_row", [H, 1, C], F8, kind="ExternalInput")
    nc.dram_tensor("v_b_row", [H, 1, C], F8, kind="ExternalInput")
    nc.dram_tensor("proj_b_col", [H, 128, KT], F32, kind="ExternalInput")
    nc.dram_tensor("fc_out_wT", [128, KT, C], F8, kind="ExternalInput")
    nc.dram_tensor("fc_out_b_row", [1, C], F8, kind="ExternalInput")
    nc.dram_tensor("offs", [1, 2], I32, kind="ExternalInput")
    nc.dram_tensor("can_in", [H, 64], F8, kind="ExternalInput")
    pe = nc.dram_tensor("pe", [S_OWN, C], BF16, kind="ExternalInput")
    out = nc.dram_tensor("out", [S_OWN, C], F32, kind="ExternalOutput")
    nc.dram_tensor("canary_out", [H, 64], F8, kind="ExternalOutput")
    with tile.TileContext(nc) as tc:
        with tc.tile_pool(name="p", bufs=2) as p:
            for ss in range(NT_OWN):
                t = p.tile([128, C], BF16, tag="t")
                nc.sync.dma_start(t[:], pe[ss * 128:(ss + 1) * 128, :])
                t2 = p.tile([128, C], F32, tag="t2")
                nc.vector.tensor_copy(t2[:], t[:])
                nc.sync.dma_start(out[ss * 128:(ss + 1) * 128, :], t2[:])
    nc.compile()
    return nc


def _pose_enc_np(s, f):
    pos = np.arange(s, dtype=np.float32)[:, None]
    div = (1.0 / (1000.0 ** (2.0 * np.arange(f, dtype=np.float32) / np.float32(f))))[None, :]
    p = np.zeros((s, f), np.float32)
    p[0::2, :] = np.sin(pos[0::2] * div)
    p[1::2, :] = np.cos(pos[1::2] * div)
    return p


def _f8(a, scale=1.0):
    return np.ascontiguousarray((np.asarray(a, np.float32) * scale).astype(NP8))


def _tile_kt(mat):
    """[C_in, N] -> [128, KT, N] (c_in = kt*128 + partition)."""
    cin, n = mat.shape
    return np.ascontiguousarray(mat.reshape(KT, 128, n).transpose(1, 0, 2))


def prepare_in_maps(x, fc_in_w, fc_in_b, ln_g, ln_b, qkv_w, qkv_b, proj_w, proj_b,
                    fc_out_w, fc_out_b):
    x = np.asarray(x, np.float32)
    qkv_w = np.asarray(qkv_w, np.float32)
    qkv_b = np.asarray(qkv_b, np.float32)
    proj_w = np.asarray(proj_w, np.float32)
    proj_b = np.asarray(proj_b, np.float32)

    shared = {
        "fc_in_wT": _tile_kt(_f8(np.asarray(fc_in_w, np.float32).T, WS)),
        "fc_in_b_row": _f8(np.asarray(fc_in_b)[None, :], WS),
        "ln_g_row": np.ascontiguousarray(np.asarray(ln_g, np.float32)[None, :]),
        "ln_b_row": np.ascontiguousarray(np.asarray(ln_b, np.float32)[None, :]),
        "wq_t": np.stack([_tile_kt(_f8(qkv_w[i, 0:C, :].T, WS)) for i in range(H)]),
        "wk_t": np.stack([_tile_kt(_f8(qkv_w[i, C:2 * C, :].T, WS)) for i in range(H)]),
        "wv_t": np.stack([_tile_kt(_f8(qkv_w[i, 2 * C:, :].T, WS)) for i in range(H)]),
        "wp_t": np.stack([_tile_kt(_f8(proj_w[i].T, WS)) for i in range(H)]),
        "q_b_col": np.ascontiguousarray(
            qkv_b[:, 0:C].reshape(H, KT, 128).transpose(0, 2, 1)),
        "k_b_row": _f8(qkv_b[:, C:2 * C][:, None, :], WS),
        "v_b_row": _f8(qkv_b[:, 2 * C:][:, None, :], WS),
        "proj_b_col": np.ascontiguousarray(
            proj_b.reshape(H, KT, 128).transpose(0, 2, 1)),
        "fc_out_wT": _tile_kt(_f8(np.asarray(fc_out_w, np.float32).T, WS)),
        "fc_out_b_row": _f8(np.asarray(fc_out_b)[None, :], WS),
    }
    pe_full = _pose_enc_np(S, C)
    in_maps = []
    for core in range(N_CORES):
        b, half = divmod(core, 2)
        own = x[b, half * S_OWN:(half + 1) * S_OWN, :].T  # [C, S_OWN]
        m = dict(shared)
        m["x_t"] = _tile_kt(_f8(own))
        m["pe"] = np.ascontiguousarray(
            pe_full[half * S_OWN:(half + 1) * S_OWN, :].astype(NPBF))
        slot_elems = 128 * (KT * C + 64)
        m["offs"] = np.array([[half * slot_elems, (1 - half) * slot_elems]], np.int32)
        m["can_in"] = np.stack(
            [np.full((64,), float((core + 1) * (2 ** i)), NP8) for i in range(H)])
        in_maps.append(m)
    return in_maps


_NC_CACHE = {}


def get_nc(n_heads=H, with_bias=True, with_ln_affine=True):
    key = (n_heads, with_bias, with_ln_affine)
    if key not in _NC_CACHE:
        _NC_CACHE[key] = build(n_heads, with_bias, with_ln_affine)
    return _NC_CACHE[key]


_EXEC_CACHE = {}


def _get_executable(nc):
    """One jitted collectives executable per process (loading a second one
    hangs the axon worker); reused across kernel() calls."""
    key = id(nc)
    if key in _EXEC_CACHE:
        return _EXEC_CACHE[key]
    import jax
    from jax.sharding import Mesh, PartitionSpec, NamedSharding
    from jax.experimental.shard_map import shard_map
    from concourse import bass2jax
    import concourse.mybir as mybir_

    bass2jax.install_neuronx_cc_hook()
    partition_name = nc.partition_id_tensor.name if nc.partition_id_tensor else None
    in_names, out_names, out_avals, zero_outs = [], [], [], []
    for alloc in nc.m.functions[0].allocations:
        if not isinstance(alloc, mybir_.MemoryLocationSet):
            continue
        name = alloc.memorylocations[0].name
        if alloc.kind == "ExternalInput":
            if name != partition_name:
                in_names.append(name)
        elif alloc.kind == "ExternalOutput":
            out_names.append(name)
            shape = tuple(alloc.tensor_shape)
            dtype = mybir_.dt.np(alloc.dtype)
            out_avals.append(jax.core.ShapedArray(shape, dtype))
            zero_outs.append(np.zeros(shape, dtype))
    n_params = len(in_names)
    n_outs = len(out_avals)
    all_in = in_names + out_names + ([partition_name] if partition_name else [])
    donate = tuple(range(n_params, n_params + n_outs))

    def _body(*args):
        operands = list(args)
        if partition_name is not None:
            operands.append(bass2jax.partition_id_tensor())
        return tuple(bass2jax._bass_exec_p.bind(
            *operands, out_avals=tuple(out_avals), in_names=tuple(all_in),
            out_names=tuple(out_names), lowering_input_output_aliases=(),
            sim_require_finite=True, sim_require_nnan=True, nc=nc))

    devices = jax.devices()[:N_CORES]
    mesh = Mesh(np.asarray(devices), ("core",))
    sharded = jax.jit(
        shard_map(_body, mesh=mesh,
                  in_specs=(PartitionSpec("core"),) * (n_params + n_outs),
                  out_specs=(PartitionSpec("core"),) * len(out_names),
                  check_rep=False),
        donate_argnums=donate, keep_unused=True)
    sh = NamedSharding(mesh, PartitionSpec("core"))
    entry = (sharded, sh, in_names[:n_params], out_names, out_avals, zero_outs)
    _EXEC_CACHE[key] = entry
    return entry


def flags_for(inputs):
    with_bias = not (np.all(np.asarray(inputs["fc_in_b"]) == 0)
                     and np.all(np.asarray(inputs["qkv_b"]) == 0)
                     and np.all(np.asarray(inputs["proj_b"]) == 0)
                     and np.all(np.asarray(inputs["fc_out_b"]) == 0))
    with_ln = not (np.all(np.asarray(inputs["ln_g"]) == 1)
                   and np.all(np.asarray(inputs["ln_b"]) == 0))
    return with_bias, with_ln


_WARMED = set()


def canaries_ok(out_arrs, out_names, out_avals):
    """True iff every core read its partner's per-head canary in every head:
    proves each head's exchange (including the early-barrier race) was clean
    for that execution."""
    ci = out_names.index("canary_out")
    pc = np.asarray(out_arrs[ci]).reshape(N_CORES, *out_avals[ci].shape)
    vals = pc.astype(np.float32)
    for core in range(N_CORES):
        for i in range(vals.shape[1]):
            if not np.all(vals[core, i] == float(((core ^ 1) + 1) * (2 ** i))):
                return False
    return True


def kernel(**inputs) -> np.ndarray:
    with_bias, with_ln = flags_for(inputs)
    nc = get_nc(H, with_bias, with_ln)
    in_maps = prepare_in_maps(**inputs)
    import jax
    sharded, sh, in_names, out_names, out_avals, zero_outs = _get_executable(nc)
    concat_in = [jax.device_put(
        np.concatenate([np.asarray(in_maps[c][nm]) for c in range(N_CORES)], axis=0), sh)
        for nm in in_names]

    def one_call():
        concat_zeros = [jax.device_put(
            np.zeros((N_CORES * z.shape[0], *z.shape[1:]), z.dtype), sh)
            for z in zero_outs]
        out_arrs = sharded(*concat_in, *concat_zeros)
        jax.block_until_ready(out_arrs)
        return out_arrs

    # The first executions after NEFF load race DGE descriptor generation
    # against the dynamic-offset register loads; registers persist across
    # executions, so retry until the canaries prove the exchange addressed
    # the right slots (typically clean from the 2nd execution).
    if id(nc) not in _WARMED:
        one_call()
        _WARMED.add(id(nc))
    for _attempt in range(8):
        out_arrs = one_call()
        oi_ = out_names.index("out")
        pc_ = np.asarray(out_arrs[oi_])
        if canaries_ok(out_arrs, out_names, out_avals) and not np.isnan(pc_).any():
            break
    oi = out_names.index("out")
    per_core = np.asarray(out_arrs[oi]).reshape(N_CORES, *out_avals[oi].shape)
    out_full = np.empty((B, S, C), np.float32)
    for core in range(N_CORES):
        b, half = divmod(core, 2)
        out_full[b, half * S_OWN:(half + 1) * S_OWN, :] = per_core[core]
    return out_full
